# revision 1
# baseline (speedup 1.0000x reference)
"""AMIP router kernel for 8 TRN2 NeuronCores (Bass/Tile, SPMD data-parallel).

Strategy
--------
B*M = 2048 masked positions are sharded 256 per core (batch-major), weights
replicated, zero collectives.  Routing softmaxes / combine weights / gathers
and the small Bm = hm@W1b + b1 term (<4% of FLOPs combined) run on host; the
device runs the heavy expert MLPs over all 2560 tokens/core:

    delta[m] = sum_{w,i} s[m,w,i] * (gelu(ha_{m,w}@W1a_i + Bm_{m,i}) @ W2_i)

with s = combine_weight * router_weight folded into one per-token-per-expert
scalar (s is exactly 0 at invalid neighbors, reproducing the reference's
masking), and the concat factored: [ha,hm]@W1 = ha@W1a + Bm.  All 160
(expert x w-block) matmul-2 products accumulate directly in PSUM, so the
per-core output is just delta^T [1024, 256] -- the final scatter into the
zero [B,S,D] tensor happens on host.

Layouts are feature-major ([feature_partition, token_free]) so both matmuls
chain without transposes.  Compute dtype bf16 (fp32 PSUM accumulate):
TensorE runs 1 cycle/row vs 2 for fp32, and the rel-err stays ~3e-3.

This walrus build enforces tiny per-instruction sync-wait budgets (DVE
tensor ops and 3-source activations: ONE wait; 2-source ACT copies and
matmuls: two; DMAs: one engine wait).  The kernel is choreographed to that
budget: per-engine program-order chaining via ordering-only dep edges, tiny
DVE "observer" copies and a self-chained ACT probe that advance each
engine's observed vector clock so Tile elides all but one wait per op, all
input tiles SBUF-resident (DMA slot reuse creates multi-queue WAW waits),
and a patched kernel-tail drain split into single-wait drains.
"""

import sys

for _p in ("/opt/trn_rl_repo",):
    if _p not in sys.path:
        sys.path.insert(0, _p)

import numpy as np
import ml_dtypes

# Problem constants (hardcoded per task spec).
B, S, D, M, K, R = 4, 2048, 1024, 512, 8, 5
W = 2 * R                 # neighbor window size (10)
D4 = D // 4               # expert hidden (256)
NCORES = 8
MC = (B * M) // NCORES    # masked positions per core (256)
T = W * MC                # device tokens per core (2560), w-major order
NBLK = 5                  # 512-wide token blocks per h-tile (T/512)
CD = D // 128             # contraction chunks over D (8)
W1A_OFF, BM_OFF, W2_OFF, SBC_OFF = 0, 2048, 3072, 5120
PK = SBC_OFF + NBLK * 512  # packed per-expert columns (7680)
BF16 = ml_dtypes.bfloat16

_COMPILED = {}            # cache: built Bass graph (shape-only, no data baked)
LAST_RESULT = None        # BassKernelResults of the most recent run
TRACE = False             # set True (e.g. from test.py) to profile


def _patch_tail_drain():
    """Split Tile's kernel-tail drain into several drains with <=4 sem waits
    each -- this walrus build rejects the single 11-wait drain the stock
    _drain_and_barrier emits for a kernel touching all 8 HW DMA queues."""
    import concourse.tile as tile
    from concourse.vector_clock import ScopedClock, VectorClock

    if getattr(tile.TileContext, "_tail_drain_patched", False):
        return

    def _drain_and_barrier(self, tick_clock, wait_clock):
        g = tick_clock.global_clock
        n = len(g)
        ticks = [g[i] for i in range(n)]
        nz = [i for i, t in enumerate(ticks) if t > 0]
        CH = 1
        for j in range(0, len(nz), CH):
            keep = set(nz[j : j + CH])
            sub = VectorClock([ticks[i] if i in keep else 0 for i in range(n)])
            d = self.nc.sync.drain()
            wait_clock.add_sem_waits(d.ins, ScopedClock({None: sub}))
        if not nz:
            d = self.nc.sync.drain()
            wait_clock.add_sem_waits(
                d.ins, ScopedClock({None: tick_clock.global_clock})
            )
        self.nc.all_engine_barrier()
        assert self.sems is not None
        popped = self.nc._tile_sem_poison_stack.pop()
        assert popped is self._sem_poison
        self.nc.clear_and_free_semaphores(list(self.sems.allocated().values()))
        self.nc.all_engine_barrier()

    tile.TileContext._drain_and_barrier = _drain_and_barrier
    tile.TileContext._tail_drain_patched = True


def _build_nc():
    import concourse.bass as bass
    import concourse.mybir as mybir
    import concourse.tile as tile
    from contextlib import ExitStack

    _patch_tail_drain()

    bf = mybir.dt.bfloat16
    f32 = mybir.dt.float32
    AF = mybir.ActivationFunctionType

    nc = bass.Bass()
    # DRAM parameters (per-core shards; all pre-laid-out [partition, free]).
    xa = nc.declare_dram_parameter("xa", [128, CD, T], bf, isOutput=False)
    # packed per-expert: [w1a (CD*D4) | bm (2*512) | w2 (2*D) | sbc (NBLK*512)]
    wpk = nc.declare_dram_parameter("wpk", [K, 128, PK], bf, isOutput=False)
    out = nc.declare_dram_parameter("out", [128, 8 * MC], f32, isOutput=True)

    with ExitStack() as ctx:
        tc = ctx.enter_context(tile.TileContext(nc))
        const = ctx.enter_context(tc.tile_pool(name="const", bufs=1))
        work = ctx.enter_context(tc.tile_pool(name="work", bufs=2))
        pd = ctx.enter_context(tc.tile_pool(name="pd", bufs=1, space="PSUM"))
        ph = ctx.enter_context(tc.tile_pool(name="ph", bufs=2, space="PSUM"))

        # Everything is resident in SBUF for the whole kernel -- no tile-slot
        # reuse for DMA'd inputs.  (Reused DMA slots create WAW deps against
        # the previous DMA's fanned-out HW queues, blowing the per-instruction
        # sync-wait slot budget in walrus.)
        # Per-engine program-order chaining (ordering-only edges): the
        # scheduler otherwise reorders ready instructions, which breaks the
        # carefully sequenced "observed clock" math that keeps every
        # instruction within its ISA struct's sync-wait budget.
        _last = {}

        def chain(instr, eng):
            if instr is None or not hasattr(instr, "ins"):
                return instr
            prev = _last.get(eng)
            if prev is not None:
                tile.add_dep_helper(
                    instr.ins, prev.ins, sync=False, reason="program-order"
                )
            _last[eng] = instr
            return instr

        # Stage xa: the first 512-token slice of every chunk lands first so
        # the first matmul block can start ~10us earlier; the tail follows.
        xa_sb = const.tile([128, CD, T], bf, tag="xa")
        nc.sync.dma_start(xa_sb[:, :, 0:512], xa[:, :, 0:512])
        # Explicit zero bias for Gelu: a float bias would be lowered to a
        # framework const AP whose init adds a second sync wait -- over the
        # 3-source Activation struct's budget of one.  DVE-owned zeros let
        # the bias dep consolidate with the DVE data dep into one wait.
        zcol = const.tile([128, 1], f32, tag="zcol")
        chain(nc.vector.memset(zcol[:], 0.0), "dve")
        # Self-chained ACT probe: waiting on its own semaphore advances the
        # scalar engine's observed self-clock, so each gelu's WAW wait
        # against the slot-recycled previous gelu is elided (the 3-source
        # Activation struct only has one sync-wait slot, needed for DVE).
        dummy_act = const.tile([1, 1], f32, tag="dummy_act")
        chain(nc.vector.memset(dummy_act[:], 0.0), "dve")
        # Warm the gelu activation-table load (~2.7us) during the input DMA
        # window instead of on the first real gelu.
        warm_t = const.tile([1, 1], f32, tag="warm_t")
        chain(
            nc.scalar.activation(
                warm_t[:], zcol[0:1, :], AF.Gelu, bias=zcol[0:1, :]
            ),
            "act",
        )
        # DVE observer scratch: tiny copies that advance VectorE's observed
        # clocks of other engines so real DVE ops carry a single sync wait
        # (this walrus build allows only ONE wait on DVE TT/Copy structs).
        scr1 = const.tile([1, 1], bf, tag="scr1")
        scr2 = const.tile([1, 512], bf, tag="scr2")
        # PE warm-up: ~20 rank-1 matmuls (~4us of PE activity) during the
        # input-DMA window keep the HAM clock gate from starting the real
        # matmul stream at half rate.  Dedicated source tile so no real
        # consumer inherits a WAR dep against the warm matmuls.
        warm_src = const.tile([1, 512], bf, tag="warm_src")
        chain(nc.vector.memset(warm_src[:], 0.0), "dve")
        warm_ps = pd.tile([128, 512], f32, tag="warm_ps", name="warm_ps")
        for wk in range(20):
            chain(nc.tensor.matmul(
                warm_ps[:],
                warm_src[0:1, 0:128],
                warm_src[0:1, :],
                start=(wk == 0),
                stop=(wk == 19),
                skip_group_check=True,
            ), "pe")

        wpk_all = const.tile([128, K, PK], bf, tag="wpk_all")
        for i in range(K):
            if i == 0:
                # expert 0 split by component (first-use order) -- its w1a is
                # on the critical path; a single packed DMA would gate the
                # first matmul on the whole 280KB transfer.
                for lo, hi in (
                    (W1A_OFF, W1A_OFF + 1024),
                    (BM_OFF, W2_OFF),
                    (SBC_OFF, PK),
                    (W2_OFF, SBC_OFF),
                    (W1A_OFF + 1024, BM_OFF),
                ):
                    nc.sync.dma_start(wpk_all[:, 0, lo:hi], wpk[0, :, lo:hi])
            else:
                nc.sync.dma_start(wpk_all[:, i], wpk[i])
            if i == 0:
                for blk in range(1, NBLK):
                    nc.sync.dma_start(
                        xa_sb[:, :, blk * 512 : blk * 512 + 512],
                        xa[:, :, blk * 512 : blk * 512 + 512],
                    )

        # Output accumulator in PSUM: delta^T [1024, 256] as 4 banks of
        # [128, 512], each holding two 128-row d-chunks side by side.
        delta_ps = [
            pd.tile([128, 512], f32, tag=f"d{j}", name=f"delta_ps{j}")
            for j in range(4)
        ]

        # Software-pipeline matmul2 one step behind matmul1 so the PE never
        # head-of-line blocks on the V->S->V gelu chain.
        pending = []  # [(i, ht, blk, hw_tile)]

        def emit_mm2(p):
            i_, ht_, blk_, hw_ = p
            w2_p = wpk_all[:, i_]
            first = i_ == 0 and ht_ == 0 and blk_ == 0
            last = i_ == K - 1 and ht_ == 1 and blk_ == NBLK - 1
            for dt in range(8):
                sl = delta_ps[dt // 2][:, (dt % 2) * 256 : (dt % 2) * 256 + 256]
                for g in range(2):
                    # start=True clears has_written for the WHOLE BANK, and
                    # each bank holds two dt regions -- so only the first
                    # region of each bank may issue start.  The second
                    # region's first write overwrites (bits cleared by the
                    # bank's single start) and accumulates thereafter.
                    chain(nc.tensor.matmul(
                        sl,
                        w2_p[:, W2_OFF + ht_ * D + dt * 128 : W2_OFF + ht_ * D + dt * 128 + 128],
                        hw_[:, g * 256 : g * 256 + 256],
                        start=(first and g == 0 and dt % 2 == 0),
                        stop=(last and g == 1),
                        skip_group_check=True,
                    ), "pe")

        for i in range(K):
            wp = wpk_all[:, i]

            # Bm_i (= hm @ W1b_i + b1_i, already w-replicated) is computed on
            # host (~3% of FLOPs) and staged via ScalarE so the DVE add sees
            # an ACT producer (single-wait budget on DVE TT ops).
            bm_sb = work.tile([128, 1024], bf, tag="bm_sb")
            chain(nc.scalar.copy(bm_sb[:], wp[:, BM_OFF : BM_OFF + 1024]), "act")

            first_of_expert = True
            for ht in range(2):
                for blk in range(NBLK):
                    hid_ps = ph.tile([128, 512], f32, tag="hid")
                    for c in range(CD):
                        chain(nc.tensor.matmul(
                            hid_ps[:],
                            wp[:, W1A_OFF + ht * 1024 + c * 128 : W1A_OFF + ht * 1024 + c * 128 + 128],
                            xa_sb[:, c, blk * 512 : blk * 512 + 512],
                            start=(c == 0),
                            stop=(c == CD - 1),
                            skip_group_check=True,
                        ), "pe")
                    if pending:
                        emit_mm2(pending.pop(0))
                        # obs1: advance DVE's observed self-clock past the
                        # previous mult, so the next mult's slot WAW is free.
                        chain(nc.vector.tensor_copy(scr1[:], hw_t[0:1, 0:1]), "dve")
                    # obs2: advance DVE's observed ACT clock past whatever
                    # ACT-produced tile the next DVE ops consume.
                    if first_of_expert:
                        chain(nc.vector.tensor_copy(scr2[:], bm_sb[0:1, 0:512]), "dve")
                        first_of_expert = False
                    else:
                        chain(nc.vector.tensor_copy(scr2[:], sbc_v[0:1, :]), "dve")
                    tmp = work.tile([128, 512], bf, tag="tmp", bufs=3)
                    chain(nc.vector.tensor_add(tmp[:], hid_ps[:], bm_sb[:, ht * 512 : ht * 512 + 512]), "dve")
                    # probe: glues ACT to this iteration; its DVE wait makes
                    # the in-place gelu need no further waits.
                    chain(nc.scalar.mul(dummy_act[:], tmp[0:1, 0:1], 0.0), "act")
                    chain(nc.scalar.activation(tmp[:], tmp[:], AF.Gelu, bias=zcol[:]), "act")
                    # ScalarE stages sbc from its DMA'd tile: DVE instructions
                    # cannot carry a DMA-queue wait at all here.
                    sbc_v = work.tile([128, 512], bf, tag="sbc_v")
                    chain(nc.scalar.copy(
                        sbc_v[:], wp[:, SBC_OFF + blk * 512 : SBC_OFF + blk * 512 + 512]
                    ), "act")
                    hw_t = work.tile([128, 512], bf, tag="hw", bufs=3)
                    chain(nc.vector.tensor_mul(hw_t[:], tmp[:], sbc_v[:]), "dve")
                    pending.append((i, ht, blk, hw_t))
        for p in pending:
            emit_mm2(p)

        # PSUM -> SBUF -> DRAM.
        delta_sb = const.tile([128, 4, 512], f32, tag="dsb")
        for j in range(4):
            if j % 2 == 0:
                chain(nc.scalar.copy(delta_sb[:, j, :], delta_ps[j][:]), "act")
            else:
                chain(
                    nc.vector.tensor_copy(delta_sb[:, j, :], delta_ps[j][:]),
                    "dve",
                )
            # SWDGE for outputs: the HW queues all have prior traffic, and a
            # queue-FIFO self-wait + the ACT data wait exceeds the DMA
            # struct's single sync-wait slot.
            nc.gpsimd.dma_start(out[:, j * 512 : j * 512 + 512], delta_sb[:, j, :])
    return nc


def _softmax(x, axis=-1):
    x = x - x.max(axis=axis, keepdims=True)
    e = np.exp(x)
    return e / e.sum(axis=axis, keepdims=True)


def kernel(h_L, mask_indices, unmasked_indices, range_r, Wr, br, W1, b1, W2, b2):
    global LAST_RESULT
    from concourse.bass_utils import run_bass_kernel_spmd

    h_L = np.asarray(h_L, np.float32)
    mask_indices = np.asarray(mask_indices, np.int32)
    unmasked_indices = np.asarray(unmasked_indices, np.int32)
    Wr, br = np.asarray(Wr, np.float32), np.asarray(br, np.float32)
    W1, b1 = np.asarray(W1, np.float32), np.asarray(b1, np.float32)
    W2, b2 = np.asarray(W2, np.float32), np.asarray(b2, np.float32)
    assert int(range_r) == R and h_L.shape == (B, S, D)

    # ---- host: gathers, masks, routing/combine softmaxes ----
    offs = np.concatenate([np.arange(-R, 0), np.arange(1, R + 1)])  # [W]
    a = mask_indices                                                # [B,M]
    t = a[:, :, None] + offs[None, None, :]                         # [B,M,W]
    in_range = (t >= 0) & (t < S)
    tcl = np.clip(t, 0, S - 1)
    is_un = np.zeros((B, S), bool)
    is_un[np.arange(B)[:, None], unmasked_indices] = True
    valid = in_range & is_un[np.arange(B)[:, None, None], tcl]      # [B,M,W]

    bidx = np.arange(B)[:, None]
    h_mask = h_L[bidx, a]                                           # [B,M,D]
    h_anchor = h_L[np.arange(B)[:, None, None], tcl]                # [B,M,W,D]

    wr = _softmax(h_mask @ Wr + br, axis=-1)                        # [B,M,K]
    scores = np.einsum("bmwd,bmd->bmw", h_anchor, h_mask) / np.sqrt(
        np.float32(D)
    )
    scores = np.where(valid, scores, np.float32(-1e30))
    cw = _softmax(scores, axis=-1) * valid                          # [B,M,W]
    s = cw[:, :, :, None] * wr[:, :, None, :]                       # [B,M,W,K]

    # ---- build per-core shards ----
    # W1 split + pre-transposed chunk layouts.
    W1a = W1[:, :D, :]                                              # [K,D,D4]
    W1b = W1[:, D:, :]
    # ht-major columns (ht*1024 + c*128 + h) so expert-0's first h-tile
    # needs only the first half of the w1a transfer.
    w1a_l = np.ascontiguousarray(
        W1a.reshape(K, CD, 128, 2, 128).transpose(0, 2, 3, 1, 4)
    ).astype(BF16)                                                  # [K,128,2,CD,128]
    w2_l = np.ascontiguousarray(
        W2.reshape(K, 2, 128, D).transpose(0, 2, 1, 3)
    ).astype(BF16)                                                  # [K,128,2,D]
    # Bm = h_mask @ W1b + b1 computed on host (~3% of FLOPs), saves device
    # matmuls.  [B,M,K,D4]
    Bm_h = np.einsum("bmd,kdh->bmkh", h_mask, W1b) + b1[None, None]

    in_maps = []
    for c in range(NCORES):
        b = c // 2
        ms = (c % 2) * MC
        ha_c = h_anchor[b, ms : ms + MC]                            # [MC,W,D]
        # tokens w-major: [W,MC,D] -> [T,D] -> transpose [D,T]
        xaT = ha_c.transpose(1, 0, 2).reshape(T, D).T               # [D,T]
        xa_l = np.ascontiguousarray(
            xaT.reshape(CD, 128, T).transpose(1, 0, 2)
        ).astype(BF16)                                              # [128,CD,T]
        # Bm^T per expert/h-tile, replicated x2 along free to match the
        # 512-wide (two w-group) blocks: [K,128,2,512]
        bm_c = Bm_h[b, ms : ms + MC]                                # [MC,K,D4]
        bmT = bm_c.transpose(1, 2, 0).reshape(K, 2, 128, MC)        # [K,ht,128,MC]
        bm_l = np.ascontiguousarray(
            np.broadcast_to(
                bmT.transpose(0, 2, 1, 3)[:, :, :, None, :],
                (K, 128, 2, 2, MC),
            ).reshape(K, 128, 2, 512)
        ).astype(BF16)
        # s tokens w-major per expert: [MC,W,K] -> [K, W*MC]
        s_c = s[b, ms : ms + MC]                                    # [MC,W,K]
        s_tok = s_c.transpose(2, 1, 0).reshape(K, 1, NBLK * 512)
        sbc_l = np.broadcast_to(s_tok, (K, 128, NBLK * 512)).astype(BF16)
        wpk_l = np.concatenate(
            [
                w1a_l.reshape(K, 128, CD * D4),
                bm_l.reshape(K, 128, 1024),
                w2_l.reshape(K, 128, 2 * D),
                sbc_l,
            ],
            axis=2,
        )
        in_maps.append(dict(xa=xa_l, wpk=wpk_l))

    key = "nc"
    if key not in _COMPILED:
        _COMPILED[key] = _build_nc()
    nc = _COMPILED[key]

    res = run_bass_kernel_spmd(
        nc, in_maps, core_ids=list(range(NCORES)), trace=TRACE
    )
    LAST_RESULT = res

    # ---- host: unshard + b2 correction + scatter ----
    delta_h = np.zeros((B, S, D), np.float32)
    sw = cw.sum(-1)                                                 # [B,M]
    corr = (sw[:, :, None] * (wr @ b2)).astype(np.float32)          # [B,M,D]
    for c in range(NCORES):
        b = c // 2
        ms = (c % 2) * MC
        o = res.results[c]["out"]                                   # [128, 8*MC]
        dT = o.reshape(128, 8, MC).transpose(1, 0, 2).reshape(D, MC)
        delta = dT.T + corr[b, ms : ms + MC]                        # [MC,D]
        delta_h[b, a[b, ms : ms + MC]] = delta
    return delta_h



# revision 10
# speedup vs baseline: 1.2108x; 1.2108x over previous
"""AMIP router kernel for 8 TRN2 NeuronCores (Bass/Tile, SPMD data-parallel).

Strategy
--------
B*M = 2048 masked positions are sharded 256 per core (batch-major), weights
replicated, zero collectives.  Routing softmaxes / combine weights / gathers
and the small Bm = hm@W1b + b1 term (<4% of FLOPs combined) run on host; the
device runs the heavy expert MLPs over all 2560 tokens/core.

Since W2 is linear, the neighbor-window sum moves BEFORE matmul-2:

    G_i[h, m]   = wr_i[m] * sum_w cw[m, w] * gelu(ha_{m,w}@W1a_i + Bm_{m,i})
    delta^T     = sum_i W2_i^T @ G_i

so matmul-2 runs on 256 columns per (expert, ht) instead of 2560 -- 10x
less PE work than the baseline that accumulated all per-token products in
PSUM.  The expert-independent combine weight cw scales each token on DVE,
Pool (otherwise idle) accumulates the 10 w-groups into a [128, 512]
running sum (two w-parity halves), and DVE folds + scales by the router
weight wr_i into the [128, 256] matmul-2 rhs.  s = cw*wr factoring also
drops the per-expert broadcast of s from the input stream (-4.6MB DMA).

Layouts are feature-major ([feature_partition, token_free]) so both matmuls
chain without transposes.  Compute dtype bf16 (fp32 PSUM accumulate).

This walrus build enforces tiny per-instruction sync-wait budgets (DVE
tensor ops and 3-source activations: ONE wait; 2-source ACT copies and
matmuls: two; DMAs: one engine wait).  The kernel is choreographed to that
budget: per-engine program-order chaining via ordering-only dep edges, tiny
DVE "observer" copies that advance each engine's observed vector clock so
Tile elides all but one wait per op, all input tiles SBUF-resident, and a
patched kernel-tail drain split into single-wait drains.
"""

import sys

for _p in ("/opt/trn_rl_repo",):
    if _p not in sys.path:
        sys.path.insert(0, _p)

import numpy as np
import ml_dtypes

# Problem constants (hardcoded per task spec).
B, S, D, M, K, R = 4, 2048, 1024, 512, 8, 5
W = 2 * R                 # neighbor window size (10)
D4 = D // 4               # expert hidden (256)
NCORES = 8
MC = (B * M) // NCORES    # masked positions per core (256)
T = W * MC                # device tokens per core (2560), w-major order
NBLK = 5                  # 512-wide token blocks per h-tile (T/512)
CD = D // 128             # contraction chunks over D (8)
W1A_OFF, BM_OFF, W2_OFF, WR_OFF = 0, 2048, 3072, 5120
PK = WR_OFF + 256         # packed per-expert columns (5376)
BF16 = ml_dtypes.bfloat16

_COMPILED = {}            # cache: built Bass graph (shape-only, no data baked)
LAST_RESULT = None        # BassKernelResults of the most recent run
TRACE = False             # set True (e.g. from test.py) to profile


def _patch_tail_drain():
    """Split Tile's kernel-tail drain into several drains with <=4 sem waits
    each -- this walrus build rejects the single many-wait drain the stock
    _drain_and_barrier emits for a kernel touching all 8 HW DMA queues."""
    import concourse.tile as tile
    from concourse.vector_clock import ScopedClock, VectorClock

    if getattr(tile.TileContext, "_tail_drain_patched", False):
        return

    def _drain_and_barrier(self, tick_clock, wait_clock):
        g = tick_clock.global_clock
        n = len(g)
        ticks = [g[i] for i in range(n)]
        nz = [i for i, t in enumerate(ticks) if t > 0]
        CH = 1
        for j in range(0, len(nz), CH):
            keep = set(nz[j : j + CH])
            sub = VectorClock([ticks[i] if i in keep else 0 for i in range(n)])
            d = self.nc.sync.drain()
            wait_clock.add_sem_waits(d.ins, ScopedClock({None: sub}))
        if not nz:
            d = self.nc.sync.drain()
            wait_clock.add_sem_waits(
                d.ins, ScopedClock({None: tick_clock.global_clock})
            )
        self.nc.all_engine_barrier()
        assert self.sems is not None
        popped = self.nc._tile_sem_poison_stack.pop()
        assert popped is self._sem_poison
        self.nc.clear_and_free_semaphores(list(self.sems.allocated().values()))
        self.nc.all_engine_barrier()

    tile.TileContext._drain_and_barrier = _drain_and_barrier
    tile.TileContext._tail_drain_patched = True


def _build_nc():
    import concourse.bass as bass
    import concourse.mybir as mybir
    import concourse.tile as tile
    from contextlib import ExitStack

    _patch_tail_drain()

    bf = mybir.dt.bfloat16
    f32 = mybir.dt.float32
    AF = mybir.ActivationFunctionType

    nc = bass.Bass()
    # DRAM parameters (per-core shards; all pre-laid-out [partition, free]).
    xa = nc.declare_dram_parameter("xa", [128, CD, T], bf, isOutput=False)
    # packed per-expert: [w1a (CD*D4) | bm (2*512) | w2 (2*D) | wr (256)]
    wpk = nc.declare_dram_parameter("wpk", [K, 128, PK], bf, isOutput=False)
    # combine weights, partition-broadcast, token (w-major) on free
    cwb = nc.declare_dram_parameter("cwb", [128, T], bf, isOutput=False)
    out = nc.declare_dram_parameter("out", [128, 8 * MC], f32, isOutput=True)

    with ExitStack() as ctx:
        tc = ctx.enter_context(tile.TileContext(nc))
        const = ctx.enter_context(tc.tile_pool(name="const", bufs=1))
        work = ctx.enter_context(tc.tile_pool(name="work", bufs=2))
        pd = ctx.enter_context(tc.tile_pool(name="pd", bufs=1, space="PSUM"))
        ph = ctx.enter_context(tc.tile_pool(name="ph", bufs=2, space="PSUM"))

        # Everything is resident in SBUF for the whole kernel -- no tile-slot
        # reuse for DMA'd inputs.  (Reused DMA slots create WAW deps against
        # the previous DMA's fanned-out HW queues, blowing the per-instruction
        # sync-wait slot budget in walrus.)
        # Per-engine program-order chaining (ordering-only edges): the
        # scheduler otherwise reorders ready instructions, which breaks the
        # carefully sequenced "observed clock" math that keeps every
        # instruction within its ISA struct's sync-wait budget.
        _last = {}

        def chain(instr, eng):
            if instr is None or not hasattr(instr, "ins"):
                return instr
            prev = _last.get(eng)
            if prev is not None:
                tile.add_dep_helper(
                    instr.ins, prev.ins, sync=False, reason="program-order"
                )
            _last[eng] = instr
            return instr

        # Stage xa: the first 512-token slice of every chunk lands first so
        # the first matmul block can start ~10us earlier; the tail follows.
        xa_sb = const.tile([128, CD, T], bf, tag="xa")
        nc.sync.dma_start(xa_sb[:, :, 0:512], xa[:, :, 0:512])
        # Combine weights land early: ACT stages them into per-block tiles
        # (DVE instructions cannot carry a DMA-queue wait in this walrus
        # build, so every DVE-read tile must have an ACT/DVE producer).
        cwb_sb = const.tile([128, T], bf, tag="cwb")
        nc.sync.dma_start(cwb_sb[:], cwb[:])
        # Explicit zero bias for Gelu: a float bias would be lowered to a
        # framework const AP whose init adds a second sync wait -- over the
        # 3-source Activation struct's budget of one.  DVE-owned zeros let
        # the bias dep consolidate with the DVE data dep into one wait.
        zcol = const.tile([128, 1], f32, tag="zcol")
        chain(nc.vector.memset(zcol[:], 0.0), "dve")
        # Self-chained ACT probe: waiting on its own semaphore advances the
        # scalar engine's observed self-clock, so each gelu's WAW wait
        # against the slot-recycled previous gelu is elided (the 3-source
        # Activation struct only has one sync-wait slot, needed for DVE).
        dummy_act = const.tile([1, 1], f32, tag="dummy_act")
        chain(nc.vector.memset(dummy_act[:], 0.0), "dve")
        # Warm the gelu activation-table load (~2.7us) during the input DMA
        # window instead of on the first real gelu.
        warm_t = const.tile([1, 1], f32, tag="warm_t")
        chain(
            nc.scalar.activation(
                warm_t[:], zcol[0:1, :], AF.Gelu, bias=zcol[0:1, :]
            ),
            "act",
        )
        # DVE observer scratch: tiny copies that advance VectorE's observed
        # clocks of other engines so real DVE ops carry a single sync wait
        # (this walrus build allows only ONE wait on DVE TT/Copy structs).
        scr1 = const.tile([1, 1], bf, tag="scr1")
        scr2 = const.tile([1, 512], bf, tag="scr2")
        scrp = const.tile([1, 1], bf, tag="scrp")
        # PE warm-up: ~20 rank-1 matmuls (~5us of PE activity) during the
        # input-DMA window keep the HAM clock gate from starting the real
        # matmul stream at half rate.  Dedicated source tile so no real
        # consumer inherits a WAR dep against the warm matmuls.
        warm_src = const.tile([1, 512], bf, tag="warm_src")
        chain(nc.vector.memset(warm_src[:], 0.0), "dve")
        warm_ps = pd.tile([128, 512], f32, tag="warm_ps", name="warm_ps")
        for wk in range(20):
            chain(nc.tensor.matmul(
                warm_ps[:],
                warm_src[0:1, 0:128],
                warm_src[0:1, :],
                start=(wk == 0),
                stop=(wk == 19),
                skip_group_check=True,
            ), "pe")

        # PE "touch" matmuls: rank-1 reads of a freshly DMA'd region that
        # carry the DMA-queue wait on a throwaway instruction, advancing the
        # PE's observed queue clock so the real matmuls (which also need a
        # DVE slot-WAR wait) stay within the single-wait Matmult budget.
        touch_ps = pd.tile([1, 2], f32, tag="touch_ps", name="touch_ps")

        def touch(region):
            chain(nc.tensor.matmul(
                touch_ps[0:1, 0:1],
                region,
                region,
                start=True,
                stop=True,
                skip_group_check=True,
            ), "pe")

        wpk_all = const.tile([128, K, PK], bf, tag="wpk_all")
        for i in range(K):
            if i == 0:
                # expert 0 split by component (first-use order) -- its w1a is
                # on the critical path; a single packed DMA would gate the
                # first matmul on the whole transfer.
                for lo, hi in (
                    (W1A_OFF, W1A_OFF + 1024),
                    (BM_OFF, W2_OFF),
                    (WR_OFF, PK),
                    (W1A_OFF + 1024, BM_OFF),
                    (W2_OFF, WR_OFF),
                ):
                    nc.sync.dma_start(wpk_all[:, 0, lo:hi], wpk[0, :, lo:hi])
            else:
                nc.sync.dma_start(wpk_all[:, i], wpk[i])
            if i == 0:
                for blk in range(1, NBLK):
                    nc.sync.dma_start(
                        xa_sb[:, :, blk * 512 : blk * 512 + 512],
                        xa[:, :, blk * 512 : blk * 512 + 512],
                    )

        # Stage combine weights into ACT-produced per-block tiles.
        cw_st = const.tile([128, NBLK, 512], bf, tag="cw_st")
        for blk in range(NBLK):
            chain(
                nc.scalar.copy(
                    cw_st[:, blk, :], cwb_sb[:, blk * 512 : blk * 512 + 512]
                ),
                "act",
            )

        # Output accumulator in PSUM: delta^T [1024, 256] as 4 banks of
        # [128, 512], each holding two 128-row d-chunks side by side.
        delta_ps = [
            pd.tile([128, 512], f32, tag=f"d{j}", name=f"delta_ps{j}")
            for j in range(4)
        ]

        # Software-pipeline matmul2 one group behind so the PE never
        # head-of-line blocks on the V->S->V->Pool reduction chain.
        pending = []  # [(i, ht, G_tile)]

        def emit_mm2(p):
            i_, ht_, g_ = p
            w2_p = wpk_all[:, i_]
            first = i_ == 0 and ht_ == 0
            last = i_ == K - 1 and ht_ == 1
            for dt in range(8):
                sl = delta_ps[dt // 2][:, (dt % 2) * 256 : (dt % 2) * 256 + 256]
                # start=True clears has_written for the WHOLE BANK, and
                # each bank holds two dt regions -- so only the first
                # region of each bank may issue start.  The second
                # region's first write overwrites (bits cleared by the
                # bank's single start) and accumulates thereafter.
                chain(nc.tensor.matmul(
                    sl,
                    w2_p[:, W2_OFF + ht_ * D + dt * 128 : W2_OFF + ht_ * D + dt * 128 + 128],
                    g_[:],
                    start=(first and dt % 2 == 0),
                    stop=last,
                    skip_group_check=True,
                ), "pe")

        hw_prev = None
        tmp_prev = None
        pb_prev = None
        group_no = 0
        for i in range(K):
            wp = wpk_all[:, i]

            # Bm_i (= hm @ W1b_i + b1_i, already w-replicated) is computed on
            # host (~3% of FLOPs) and staged via ScalarE so the DVE add sees
            # an ACT producer (single-wait budget on DVE TT ops).  Same for
            # the router-weight row wr_i.
            bm_sb = work.tile([128, 1024], bf, tag="bm_sb")
            chain(nc.scalar.copy(bm_sb[:], wp[:, BM_OFF : BM_OFF + 1024]), "act")
            wr_sb = work.tile([128, 256], bf, tag="wr_sb")
            chain(nc.scalar.copy(wr_sb[:], wp[:, WR_OFF : WR_OFF + 256]), "act")

            first_of_expert = True
            for ht in range(2):
                # Pool self-observer: one Pool self-wait on last group's
                # final reduce advances Pool's observed self-clock, so this
                # group's pair-adds carry only their DVE data wait (the
                # recycled-slot WAW would otherwise be a second wait).
                if pb_prev is not None:
                    chain(nc.gpsimd.tensor_copy(scrp[:], pb_prev[0:1, 0:1]), "pool")
                hw_g = []
                for blk in range(NBLK):
                    # absorb DMA first-touch waits on throwaway touch matmuls
                    if i == 0 and ht == 0:
                        touch(xa_sb[0:1, 0, blk * 512 : blk * 512 + 1])
                    if blk == 0:
                        if i == 0:
                            touch(wp[0:1, W1A_OFF + ht * 1024 : W1A_OFF + ht * 1024 + 1])
                        elif ht == 0:
                            touch(wp[0:1, 0:1])
                    hid_ps = ph.tile([128, 512], f32, tag="hid")
                    for c in range(CD):
                        chain(nc.tensor.matmul(
                            hid_ps[:],
                            wp[:, W1A_OFF + ht * 1024 + c * 128 : W1A_OFF + ht * 1024 + c * 128 + 128],
                            xa_sb[:, c, blk * 512 : blk * 512 + 512],
                            start=(c == 0),
                            stop=(c == CD - 1),
                            skip_group_check=True,
                        ), "pe")
                    if blk == 2 and pending:
                        # one group behind, and two blocks in: the V->S->V->
                        # Pool reduction chain finishes ~3.8us after the
                        # previous group's last mm1, so emitting here keeps
                        # the PE stall-free (a stall resets the p-state ramp).
                        p = pending.pop(0)
                        if p[0] == 0 and p[1] == 0:
                            # expert-0's w2 arrives as a separate DMA chunk;
                            # its queue wait rides a touch, not the matmul.
                            touch(wpk_all[0:1, 0, W2_OFF : W2_OFF + 1])
                        emit_mm2(p)
                    # obs0: a DVE self-wait on the most recent mult advances
                    # the observed self-clock, eliding every older same-engine
                    # WAW/RAW (recycled tmp slots etc).
                    if hw_prev is not None:
                        chain(nc.vector.tensor_copy(scr1[:], hw_prev[0:1, 0:1]), "dve")
                    # obs2: advance DVE's observed ACT clock past whatever
                    # ACT-produced tiles the next DVE ops consume (bm/wr/cw
                    # staging at expert start; the in-place gelu otherwise).
                    if first_of_expert:
                        chain(nc.vector.tensor_copy(scr2[:], bm_sb[0:1, 0:512]), "dve")
                        first_of_expert = False
                    else:
                        chain(nc.vector.tensor_copy(scr2[:], tmp_prev[0:1, 0:512]), "dve")
                    tmp = work.tile([128, 512], bf, tag="tmp", bufs=3)
                    chain(nc.vector.tensor_add(tmp[:], hid_ps[:], bm_sb[:, ht * 512 : ht * 512 + 512]), "dve")
                    # probe: glues ACT to this iteration; its DVE wait makes
                    # the in-place gelu need no further waits.
                    chain(nc.scalar.mul(dummy_act[:], tmp[0:1, 0:1], 0.0), "act")
                    chain(nc.scalar.activation(tmp[:], tmp[:], AF.Gelu, bias=zcol[:]), "act")
                    hw_t = work.tile([128, 512], bf, tag="hw", bufs=5)
                    chain(nc.vector.tensor_mul(hw_t[:], tmp[:], cw_st[:, blk, :]), "dve")
                    hw_g.append(hw_t)
                    hw_prev = hw_t
                    tmp_prev = tmp
                    # Pool (otherwise idle) owns the w-window reduction as a
                    # TREE: pair-adds of DVE products carry one DVE wait;
                    # the self-chained combines carry one Pool self-wait.
                    if blk == 1:
                        p_a = work.tile([128, 512], bf, tag="pa")
                        chain(nc.gpsimd.tensor_add(p_a[:], hw_g[0][:], hw_g[1][:]), "pool")
                    elif blk == 3:
                        p_b = work.tile([128, 512], bf, tag="pb")
                        chain(nc.gpsimd.tensor_add(p_b[:], hw_g[2][:], hw_g[3][:]), "pool")
                # f1 = pa + pb; f2 = f1 + hw4; g1 folds the two w-parity
                # halves -- all Pool, each one self/DVE wait.
                f1 = work.tile([128, 512], bf, tag="f1")
                chain(nc.gpsimd.tensor_add(f1[:], p_a[:], p_b[:]), "pool")
                # second Pool self-observer: f2 reads f1 (self) AND hw4 (DVE)
                # -- two waits without this; observing f1 here leaves f2 with
                # only the DVE data wait.
                chain(nc.gpsimd.tensor_copy(scrp[:], f1[0:1, 0:1]), "pool")
                f2 = work.tile([128, 512], bf, tag="f2")
                chain(nc.gpsimd.tensor_add(f2[:], f1[:], hw_g[4][:]), "pool")
                g1 = work.tile([128, 256], bf, tag="g1")
                chain(nc.gpsimd.tensor_add(g1[:], f2[:, 0:256], f2[:, 256:512]), "pool")
                # scale by the router weight on DVE: G = g1 * wr_i.
                g_t = work.tile([128, 256], bf, tag="G")
                chain(nc.vector.tensor_mul(g_t[:], g1[:], wr_sb[:]), "dve")
                pending.append((i, ht, g_t))
                pb_prev = p_b
                group_no += 1
        for p in pending:
            emit_mm2(p)

        # PSUM -> SBUF -> DRAM.
        delta_sb = const.tile([128, 4, 512], f32, tag="dsb")
        for j in range(4):
            if j % 2 == 0:
                chain(nc.scalar.copy(delta_sb[:, j, :], delta_ps[j][:]), "act")
            else:
                chain(
                    nc.vector.tensor_copy(delta_sb[:, j, :], delta_ps[j][:]),
                    "dve",
                )
            # SWDGE for outputs: the HW queues all have prior traffic, and a
            # queue-FIFO self-wait + the ACT data wait exceeds the DMA
            # struct's single sync-wait slot.
            chain(nc.gpsimd.dma_start(out[:, j * 512 : j * 512 + 512], delta_sb[:, j, :]), "pool")
    return nc


def _softmax(x, axis=-1):
    x = x - x.max(axis=axis, keepdims=True)
    e = np.exp(x)
    return e / e.sum(axis=axis, keepdims=True)


def kernel(h_L, mask_indices, unmasked_indices, range_r, Wr, br, W1, b1, W2, b2):
    global LAST_RESULT
    from concourse.bass_utils import run_bass_kernel_spmd

    h_L = np.asarray(h_L, np.float32)
    mask_indices = np.asarray(mask_indices, np.int32)
    unmasked_indices = np.asarray(unmasked_indices, np.int32)
    Wr, br = np.asarray(Wr, np.float32), np.asarray(br, np.float32)
    W1, b1 = np.asarray(W1, np.float32), np.asarray(b1, np.float32)
    W2, b2 = np.asarray(W2, np.float32), np.asarray(b2, np.float32)
    assert int(range_r) == R and h_L.shape == (B, S, D)

    # ---- host: gathers, masks, routing/combine softmaxes ----
    offs = np.concatenate([np.arange(-R, 0), np.arange(1, R + 1)])  # [W]
    a = mask_indices                                                # [B,M]
    t = a[:, :, None] + offs[None, None, :]                         # [B,M,W]
    in_range = (t >= 0) & (t < S)
    tcl = np.clip(t, 0, S - 1)
    is_un = np.zeros((B, S), bool)
    is_un[np.arange(B)[:, None], unmasked_indices] = True
    valid = in_range & is_un[np.arange(B)[:, None, None], tcl]      # [B,M,W]

    bidx = np.arange(B)[:, None]
    h_mask = h_L[bidx, a]                                           # [B,M,D]
    h_anchor = h_L[np.arange(B)[:, None, None], tcl]                # [B,M,W,D]

    wr = _softmax(h_mask @ Wr + br, axis=-1)                        # [B,M,K]
    scores = np.einsum("bmwd,bmd->bmw", h_anchor, h_mask) / np.sqrt(
        np.float32(D)
    )
    scores = np.where(valid, scores, np.float32(-1e30))
    cw = _softmax(scores, axis=-1) * valid                          # [B,M,W]

    # ---- build per-core shards ----
    # W1 split + pre-transposed chunk layouts.
    W1a = W1[:, :D, :]                                              # [K,D,D4]
    W1b = W1[:, D:, :]
    # ht-major columns (ht*1024 + c*128 + h) so expert-0's first h-tile
    # needs only the first half of the w1a transfer.
    w1a_l = np.ascontiguousarray(
        W1a.reshape(K, CD, 128, 2, 128).transpose(0, 2, 3, 1, 4)
    ).astype(BF16)                                                  # [K,128,2,CD,128]
    w2_l = np.ascontiguousarray(
        W2.reshape(K, 2, 128, D).transpose(0, 2, 1, 3)
    ).astype(BF16)                                                  # [K,128,2,D]
    # Bm = h_mask @ W1b + b1 computed on host (~3% of FLOPs), saves device
    # matmuls.  [B,M,K,D4]
    Bm_h = np.einsum("bmd,kdh->bmkh", h_mask, W1b) + b1[None, None]

    in_maps = []
    for c in range(NCORES):
        b = c // 2
        ms = (c % 2) * MC
        ha_c = h_anchor[b, ms : ms + MC]                            # [MC,W,D]
        # tokens w-major: [W,MC,D] -> [T,D] -> transpose [D,T]
        xaT = ha_c.transpose(1, 0, 2).reshape(T, D).T               # [D,T]
        xa_l = np.ascontiguousarray(
            xaT.reshape(CD, 128, T).transpose(1, 0, 2)
        ).astype(BF16)                                              # [128,CD,T]
        # Bm^T per expert/h-tile, replicated x2 along free to match the
        # 512-wide (two w-group) blocks: [K,128,2,512]
        bm_c = Bm_h[b, ms : ms + MC]                                # [MC,K,D4]
        bmT = bm_c.transpose(1, 2, 0).reshape(K, 2, 128, MC)        # [K,ht,128,MC]
        bm_l = np.ascontiguousarray(
            np.broadcast_to(
                bmT.transpose(0, 2, 1, 3)[:, :, :, None, :],
                (K, 128, 2, 2, MC),
            ).reshape(K, 128, 2, 512)
        ).astype(BF16)
        # router weights per expert, partition-broadcast: [K,128,256]
        wr_c = wr[b, ms : ms + MC]                                  # [MC,K]
        wr_l = np.broadcast_to(
            wr_c.T[:, None, :], (K, 128, MC)
        ).astype(BF16)
        # combine weights w-major, partition-broadcast: [128, T]
        cw_c = cw[b, ms : ms + MC]                                  # [MC,W]
        cwb_l = np.broadcast_to(
            cw_c.T.reshape(1, T), (128, T)
        ).astype(BF16)
        wpk_l = np.concatenate(
            [
                w1a_l.reshape(K, 128, CD * D4),
                bm_l.reshape(K, 128, 1024),
                w2_l.reshape(K, 128, 2 * D),
                wr_l,
            ],
            axis=2,
        )
        in_maps.append(dict(xa=xa_l, wpk=wpk_l, cwb=np.ascontiguousarray(cwb_l)))

    key = "nc"
    if key not in _COMPILED:
        _COMPILED[key] = _build_nc()
    nc = _COMPILED[key]

    res = run_bass_kernel_spmd(
        nc, in_maps, core_ids=list(range(NCORES)), trace=TRACE
    )
    LAST_RESULT = res

    # ---- host: unshard + b2 correction + scatter ----
    delta_h = np.zeros((B, S, D), np.float32)
    sw = cw.sum(-1)                                                 # [B,M]
    corr = (sw[:, :, None] * (wr @ b2)).astype(np.float32)          # [B,M,D]
    for c in range(NCORES):
        b = c // 2
        ms = (c % 2) * MC
        o = res.results[c]["out"]                                   # [128, 8*MC]
        dT = o.reshape(128, 8, MC).transpose(1, 0, 2).reshape(D, MC)
        delta = dT.T + corr[b, ms : ms + MC]                        # [MC,D]
        delta_h[b, a[b, ms : ms + MC]] = delta
    return delta_h


# revision 17
# speedup vs baseline: 1.4920x; 1.2322x over previous
"""AMIP router kernel for 8 TRN2 NeuronCores (Bass/Tile, SPMD data-parallel).

Strategy
--------
B*M = 2048 masked positions are sharded 256 per core (batch-major), weights
replicated, zero collectives.  Routing softmaxes / combine weights / gathers
and the small Bm = hm@W1b + b1 term (<4% of FLOPs combined) run on host; the
device runs the heavy expert MLPs over all 2560 tokens/core.

Since W2 is linear, the neighbor-window sum moves BEFORE matmul-2:

    G_i[h, m]   = wr_i[m] * sum_w cw[m, w] * gelu(ha_{m,w}@W1a_i + Bm_{m,i})
    delta^T     = sum_i W2_i^T @ G_i

so matmul-2 runs on 256 columns per (expert, ht) instead of 2560 -- 10x
less PE work than the baseline that accumulated all per-token products in
PSUM.  The expert-independent combine weight cw scales each token on DVE,
Pool (otherwise idle) accumulates the 10 w-groups into a [128, 512]
running sum (two w-parity halves), and DVE folds + scales by the router
weight wr_i into the [128, 256] matmul-2 rhs.  s = cw*wr factoring also
drops the per-expert broadcast of s from the input stream (-4.6MB DMA).

Layouts are feature-major ([feature_partition, token_free]) so both matmuls
chain without transposes.  Compute dtype bf16 (fp32 PSUM accumulate).

This walrus build enforces tiny per-instruction sync-wait budgets (DVE
tensor ops and 3-source activations: ONE wait; 2-source ACT copies and
matmuls: two; DMAs: one engine wait).  The kernel is choreographed to that
budget: per-engine program-order chaining via ordering-only dep edges, tiny
DVE "observer" copies that advance each engine's observed vector clock so
Tile elides all but one wait per op, all input tiles SBUF-resident, and a
patched kernel-tail drain split into single-wait drains.
"""

import sys

for _p in ("/opt/trn_rl_repo",):
    if _p not in sys.path:
        sys.path.insert(0, _p)

import numpy as np
import ml_dtypes

# Problem constants (hardcoded per task spec).
B, S, D, M, K, R = 4, 2048, 1024, 512, 8, 5
W = 2 * R                 # neighbor window size (10)
D4 = D // 4               # expert hidden (256)
NCORES = 8
MC = (B * M) // NCORES    # masked positions per core (256)
T = W * MC                # device tokens per core (2560), w-major order
NBLK = 5                  # 512-wide token blocks per h-tile (T/512)
CD = D // 128             # contraction chunks over D (8)
W1A_OFF, BM_OFF, W2_OFF, WR_OFF = 0, 2048, 3072, 5120
PK = WR_OFF + 256         # packed per-expert columns (5376)
BF16 = ml_dtypes.bfloat16

_COMPILED = {}            # cache: built Bass graph (shape-only, no data baked)
LAST_RESULT = None        # BassKernelResults of the most recent run
TRACE = False             # set True (e.g. from test.py) to profile


def _patch_tail_drain():
    """Split Tile's kernel-tail drain into several drains with <=4 sem waits
    each -- this walrus build rejects the single many-wait drain the stock
    _drain_and_barrier emits for a kernel touching all 8 HW DMA queues."""
    import concourse.tile as tile
    from concourse.vector_clock import ScopedClock, VectorClock

    if getattr(tile.TileContext, "_tail_drain_patched", False):
        return

    def _drain_and_barrier(self, tick_clock, wait_clock):
        g = tick_clock.global_clock
        n = len(g)
        ticks = [g[i] for i in range(n)]
        nz = [i for i, t in enumerate(ticks) if t > 0]
        CH = 1
        for j in range(0, len(nz), CH):
            keep = set(nz[j : j + CH])
            sub = VectorClock([ticks[i] if i in keep else 0 for i in range(n)])
            d = self.nc.sync.drain()
            wait_clock.add_sem_waits(d.ins, ScopedClock({None: sub}))
        if not nz:
            d = self.nc.sync.drain()
            wait_clock.add_sem_waits(
                d.ins, ScopedClock({None: tick_clock.global_clock})
            )
        self.nc.all_engine_barrier()
        assert self.sems is not None
        popped = self.nc._tile_sem_poison_stack.pop()
        assert popped is self._sem_poison
        self.nc.clear_and_free_semaphores(list(self.sems.allocated().values()))
        self.nc.all_engine_barrier()

    tile.TileContext._drain_and_barrier = _drain_and_barrier
    tile.TileContext._tail_drain_patched = True


def _build_nc():
    import concourse.bass as bass
    import concourse.mybir as mybir
    import concourse.tile as tile
    from contextlib import ExitStack

    _patch_tail_drain()

    bf = mybir.dt.bfloat16
    f32 = mybir.dt.float32
    AF = mybir.ActivationFunctionType

    nc = bass.Bass()
    # DRAM parameters (per-core shards; all pre-laid-out [partition, free]).
    xa = nc.declare_dram_parameter("xa", [128, CD, T], bf, isOutput=False)
    # packed per-expert: [w1a (CD*D4) | bm (2*512) | w2 (2*D) | wr (256)]
    wpk = nc.declare_dram_parameter("wpk", [K, 128, PK], bf, isOutput=False)
    # combine weights, partition-broadcast, token (w-major) on free
    cwb = nc.declare_dram_parameter("cwb", [128, T], bf, isOutput=False)
    out = nc.declare_dram_parameter("out", [128, 8 * MC], f32, isOutput=True)

    with ExitStack() as ctx:
        tc = ctx.enter_context(tile.TileContext(nc))
        const = ctx.enter_context(tc.tile_pool(name="const", bufs=1))
        work = ctx.enter_context(tc.tile_pool(name="work", bufs=2))
        pd = ctx.enter_context(tc.tile_pool(name="pd", bufs=1, space="PSUM"))
        ph = ctx.enter_context(tc.tile_pool(name="ph", bufs=3, space="PSUM"))

        # Everything is resident in SBUF for the whole kernel -- no tile-slot
        # reuse for DMA'd inputs.  (Reused DMA slots create WAW deps against
        # the previous DMA's fanned-out HW queues, blowing the per-instruction
        # sync-wait slot budget in walrus.)
        # Per-engine program-order chaining (ordering-only edges): the
        # scheduler otherwise reorders ready instructions, which breaks the
        # carefully sequenced "observed clock" math that keeps every
        # instruction within its ISA struct's sync-wait budget.
        _last = {}

        def chain(instr, eng):
            if instr is None or not hasattr(instr, "ins"):
                return instr
            prev = _last.get(eng)
            if prev is not None:
                tile.add_dep_helper(
                    instr.ins, prev.ins, sync=False, reason="program-order"
                )
            _last[eng] = instr
            return instr

        # Stage xa: the first 512-token slice of every chunk lands first so
        # the first matmul block can start ~10us earlier; the tail follows.
        xa_sb = const.tile([128, CD, T], bf, tag="xa")
        nc.sync.dma_start(xa_sb[:, :, 0:512], xa[:, :, 0:512])
        # Combine weights land early: ACT stages them into per-block tiles
        # (DVE instructions cannot carry a DMA-queue wait in this walrus
        # build, so every DVE-read tile must have an ACT/DVE producer).
        cwb_sb = const.tile([128, T], bf, tag="cwb")
        nc.sync.dma_start(cwb_sb[:], cwb[:])
        # Explicit zero bias for Gelu: a float bias would be lowered to a
        # framework const AP whose init adds a second sync wait -- over the
        # 3-source Activation struct's budget of one.  DVE-owned zeros let
        # the bias dep consolidate with the DVE data dep into one wait.
        zcol = const.tile([128, 1], f32, tag="zcol")
        chain(nc.vector.memset(zcol[:], 0.0), "dve")
        # Self-chained ACT probe: waiting on its own semaphore advances the
        # scalar engine's observed self-clock, so each gelu's WAW wait
        # against the slot-recycled previous gelu is elided (the 3-source
        # Activation struct only has one sync-wait slot, needed for DVE).
        dummy_act = const.tile([1, 1], f32, tag="dummy_act")
        chain(nc.vector.memset(dummy_act[:], 0.0), "dve")
        # Warm the gelu activation-table load (~2.7us) during the input DMA
        # window instead of on the first real gelu.
        warm_t = const.tile([1, 1], f32, tag="warm_t")
        chain(
            nc.scalar.activation(
                warm_t[:], zcol[0:1, :], AF.Gelu, bias=zcol[0:1, :]
            ),
            "act",
        )
        # DVE observer scratch: tiny copies that advance VectorE's observed
        # clocks of other engines so real DVE ops carry a single sync wait
        # (this walrus build allows only ONE wait on DVE TT/Copy structs).
        scr1 = const.tile([1, 1], bf, tag="scr1")
        scr2 = const.tile([1, 512], bf, tag="scr2")
        scr2e = const.tile([1, 512], bf, tag="scr2e")
        scrp = const.tile([1, 1], bf, tag="scrp")
        # PE warm-up: ~20 rank-1 matmuls (~5us of PE activity) during the
        # input-DMA window keep the HAM clock gate from starting the real
        # matmul stream at half rate.  Dedicated source tile so no real
        # consumer inherits a WAR dep against the warm matmuls.
        warm_src = const.tile([1, 512], bf, tag="warm_src")
        chain(nc.vector.memset(warm_src[:], 0.0), "dve")
        warm_ps = pd.tile([128, 512], f32, tag="warm_ps", name="warm_ps")
        for wk in range(20):
            chain(nc.tensor.matmul(
                warm_ps[:],
                warm_src[0:1, 0:128],
                warm_src[0:1, :],
                start=(wk == 0),
                stop=(wk == 19),
                skip_group_check=True,
            ), "pe")

        # PE "touch" matmuls: rank-1 reads of a freshly DMA'd region that
        # carry the DMA-queue wait on a throwaway instruction, advancing the
        # PE's observed queue clock so the real matmuls (which also need a
        # DVE slot-WAR wait) stay within the single-wait Matmult budget.
        # They overwrite a corner of the (finished) warm-up bank -- a
        # dedicated PSUM tile would cost the bank that hid_ps triple
        # buffering needs.
        def touch(region):
            chain(nc.tensor.matmul(
                warm_ps[0:1, 0:1],
                region,
                region,
                start=True,
                stop=True,
                skip_group_check=True,
            ), "pe")

        wpk_all = const.tile([128, K, PK], bf, tag="wpk_all")
        for i in range(K):
            if i == 0:
                # expert 0 split by component (first-use order) -- its w1a is
                # on the critical path; a single packed DMA would gate the
                # first matmul on the whole transfer.
                for lo, hi in (
                    (W1A_OFF, W1A_OFF + 1024),
                    (BM_OFF, W2_OFF),
                    (WR_OFF, PK),
                    (W1A_OFF + 1024, BM_OFF),
                    (W2_OFF, WR_OFF),
                ):
                    nc.sync.dma_start(wpk_all[:, 0, lo:hi], wpk[0, :, lo:hi])
            else:
                nc.sync.dma_start(wpk_all[:, i], wpk[i])
            if i == 0:
                for blk in range(1, NBLK):
                    nc.sync.dma_start(
                        xa_sb[:, :, blk * 512 : blk * 512 + 512],
                        xa[:, :, blk * 512 : blk * 512 + 512],
                    )

        # Stage combine weights into ACT-produced per-block tiles.
        cw_st = const.tile([128, NBLK, 512], bf, tag="cw_st")
        for blk in range(NBLK):
            chain(
                nc.scalar.copy(
                    cw_st[:, blk, :], cwb_sb[:, blk * 512 : blk * 512 + 512]
                ),
                "act",
            )

        # Output accumulator in PSUM: delta^T [1024, 256] as 4 banks of
        # [128, 512], each holding two 128-row d-chunks side by side.
        delta_ps = [
            pd.tile([128, 512], f32, tag=f"d{j}", name=f"delta_ps{j}")
            for j in range(4)
        ]

        # Software-pipeline matmul2 one group behind so the PE never
        # head-of-line blocks on the V->S->V->Pool reduction chain.
        pending = []  # [(i, ht, G_tile)]

        def emit_mm2(p):
            i_, ht_, g_ = p
            w2_p = wpk_all[:, i_]
            first = i_ == 0 and ht_ == 0
            last = i_ == K - 1 and ht_ == 1
            for dt in range(8):
                sl = delta_ps[dt // 2][:, (dt % 2) * 256 : (dt % 2) * 256 + 256]
                # start=True clears has_written for the WHOLE BANK, and
                # each bank holds two dt regions -- so only the first
                # region of each bank may issue start.  The second
                # region's first write overwrites (bits cleared by the
                # bank's single start) and accumulates thereafter.
                chain(nc.tensor.matmul(
                    sl,
                    w2_p[:, W2_OFF + ht_ * D + dt * 128 : W2_OFF + ht_ * D + dt * 128 + 128],
                    g_[:],
                    start=(first and dt % 2 == 0),
                    stop=last,
                    skip_group_check=True,
                ), "pe")

        hw_hist = []              # all hw tiles in DVE emission order
        pb_prev = None
        for i in range(K):
            wp = wpk_all[:, i]

            # Bm_i (= hm @ W1b_i + b1_i, already w-replicated) is computed on
            # host (~3% of FLOPs) and staged via ScalarE so the DVE add sees
            # an ACT producer (single-wait budget on DVE TT ops).  Same for
            # the router-weight row wr_i.
            bm_sb = work.tile([128, 1024], bf, tag="bm_sb")
            chain(nc.scalar.copy(bm_sb[:], wp[:, BM_OFF : BM_OFF + 1024]), "act")
            wr_sb = work.tile([128, 256], bf, tag="wr_sb")
            chain(nc.scalar.copy(wr_sb[:], wp[:, WR_OFF : WR_OFF + 256]), "act")

            first_of_expert = True
            for ht in range(2):
                # Pool self-observer: one Pool self-wait on last group's
                # final reduce advances Pool's observed self-clock, so this
                # group's pair-adds carry only their DVE data wait (the
                # recycled-slot WAW would otherwise be a second wait).
                if pb_prev is not None:
                    chain(nc.gpsimd.tensor_copy(scrp[:], pb_prev[0:1, 0:1]), "pool")
                hw_g = []
                pend_mul = []     # [(tmp, blk)] gelu issued, mul not yet

                def emit_mul():
                    """mul for the oldest gelu'd block.  Runs one block
                    behind the bias-add so the V->S->V round-trip (gelu +
                    two sem hops, ~1us) overlaps the next bias-add instead
                    of serializing the per-block pipeline."""
                    tmp_, blk_ = pend_mul.pop(0)
                    # obs2-late: waits on this block's gelu, advancing DVE's
                    # observed ACT clock so the mul itself carries no waits.
                    chain(nc.vector.tensor_copy(scr2[:], tmp_[0:1, 0:512]), "dve")
                    hw_t = work.tile([128, 512], bf, tag="hw", bufs=5)
                    chain(nc.vector.tensor_mul(hw_t[:], tmp_[:], cw_st[:, blk_, :]), "dve")
                    hw_g.append(hw_t)
                    hw_hist.append(hw_t)
                    # Pool (otherwise idle) owns the w-window reduction as a
                    # TREE: pair-adds of DVE products carry one DVE wait;
                    # the self-chained combines carry one Pool self-wait.
                    if len(hw_g) == 2:
                        pa_ = work.tile([128, 512], bf, tag="pa")
                        chain(nc.gpsimd.tensor_add(pa_[:], hw_g[0][:], hw_g[1][:]), "pool")
                        hw_g.append(pa_)  # stash (slots 2+ unused for hw)
                    return hw_t

                p_a = p_b = None
                for blk in range(NBLK):
                    # absorb DMA first-touch waits on throwaway touch matmuls
                    if i == 0 and ht == 0:
                        touch(xa_sb[0:1, 0, blk * 512 : blk * 512 + 1])
                    if blk == 0:
                        if i == 0:
                            touch(wp[0:1, W1A_OFF + ht * 1024 : W1A_OFF + ht * 1024 + 1])
                        elif ht == 0:
                            touch(wp[0:1, 0:1])
                    hid_ps = ph.tile([128, 512], f32, tag="hid")
                    for c in range(CD):
                        chain(nc.tensor.matmul(
                            hid_ps[:],
                            wp[:, W1A_OFF + ht * 1024 + c * 128 : W1A_OFF + ht * 1024 + c * 128 + 128],
                            xa_sb[:, c, blk * 512 : blk * 512 + 512],
                            start=(c == 0),
                            stop=(c == CD - 1),
                            skip_group_check=True,
                        ), "pe")
                    if blk == 2 and pending:
                        # one group behind, and two blocks in: the reduction
                        # chain finishes ~3.6us after the previous group's
                        # last mm1, so emitting here keeps the PE stall-free
                        # (a stall resets the p-state ramp).
                        p = pending.pop(0)
                        if p[0] == 0 and p[1] == 0:
                            # expert-0's w2 arrives as a separate DMA chunk;
                            # its queue wait rides a touch, not the matmul.
                            touch(wpk_all[0:1, 0, W2_OFF : W2_OFF + 1])
                        emit_mm2(p)
                    # obs0: a DVE self-wait on the latest mult advances the
                    # observed self-clock, eliding every older same-engine
                    # WAW/RAW (recycled tmp/scr2 slots etc).
                    if hw_hist:
                        chain(nc.vector.tensor_copy(scr1[:], hw_hist[-1][0:1, 0:1]), "dve")
                    # obs2-early at expert start: the bias-add below reads
                    # this expert's freshly ACT-staged bm tile.  Writes its
                    # own scratch -- sharing scr2 with obs2-late would add a
                    # WAR wait there.
                    if first_of_expert:
                        chain(nc.vector.tensor_copy(scr2e[:], bm_sb[0:1, 0:512]), "dve")
                        first_of_expert = False
                    tmp = work.tile([128, 512], bf, tag="tmp", bufs=3)
                    chain(nc.vector.tensor_add(tmp[:], hid_ps[:], bm_sb[:, ht * 512 : ht * 512 + 512]), "dve")
                    # probe: glues ACT to this iteration; its DVE wait makes
                    # the in-place gelu need no further waits.
                    chain(nc.scalar.mul(dummy_act[:], tmp[0:1, 0:1], 0.0), "act")
                    chain(nc.scalar.activation(tmp[:], tmp[:], AF.Gelu, bias=zcol[:]), "act")
                    pend_mul.append((tmp, blk))
                    if len(pend_mul) == 2:
                        hw_new = emit_mul()
                        if blk == 4:
                            p_b = work.tile([128, 512], bf, tag="pb")
                            chain(nc.gpsimd.tensor_add(p_b[:], hw_g[2 + 1][:], hw_new[:]), "pool")
                # tail: the final block's mul, then the Pool combine chain.
                chain(nc.vector.tensor_copy(scr1[:], hw_hist[-1][0:1, 0:1]), "dve")
                hw_last = emit_mul()
                p_a = hw_g[2]     # stashed pair-add of blocks 0+1
                # f1 = pa + pb; f2 = f1 + hw4; g1 folds the two w-parity
                # halves -- all Pool, each one self/DVE wait.
                f1 = work.tile([128, 512], bf, tag="f1")
                chain(nc.gpsimd.tensor_add(f1[:], p_a[:], p_b[:]), "pool")
                # second Pool self-observer: f2 reads f1 (self) AND hw4 (DVE)
                # -- two waits without this; observing f1 here leaves f2 with
                # only the DVE data wait.
                chain(nc.gpsimd.tensor_copy(scrp[:], f1[0:1, 0:1]), "pool")
                f2 = work.tile([128, 512], bf, tag="f2")
                chain(nc.gpsimd.tensor_add(f2[:], f1[:], hw_last[:]), "pool")
                g1 = work.tile([128, 256], bf, tag="g1")
                chain(nc.gpsimd.tensor_add(g1[:], f2[:, 0:256], f2[:, 256:512]), "pool")
                # scale by the router weight on DVE: G = g1 * wr_i.
                g_t = work.tile([128, 256], bf, tag="G")
                chain(nc.vector.tensor_mul(g_t[:], g1[:], wr_sb[:]), "dve")
                pending.append((i, ht, g_t))
                pb_prev = p_b
        for p in pending:
            emit_mm2(p)

        # PSUM -> SBUF -> DRAM.
        delta_sb = const.tile([128, 4, 512], f32, tag="dsb")
        for j in range(4):
            if j % 2 == 0:
                chain(nc.scalar.copy(delta_sb[:, j, :], delta_ps[j][:]), "act")
            else:
                chain(
                    nc.vector.tensor_copy(delta_sb[:, j, :], delta_ps[j][:]),
                    "dve",
                )
            # SWDGE for outputs: the HW queues all have prior traffic, and a
            # queue-FIFO self-wait + the ACT data wait exceeds the DMA
            # struct's single sync-wait slot.
            chain(nc.gpsimd.dma_start(out[:, j * 512 : j * 512 + 512], delta_sb[:, j, :]), "pool")
    return nc


def _softmax(x, axis=-1):
    x = x - x.max(axis=axis, keepdims=True)
    e = np.exp(x)
    return e / e.sum(axis=axis, keepdims=True)


def kernel(h_L, mask_indices, unmasked_indices, range_r, Wr, br, W1, b1, W2, b2):
    global LAST_RESULT
    from concourse.bass_utils import run_bass_kernel_spmd

    h_L = np.asarray(h_L, np.float32)
    mask_indices = np.asarray(mask_indices, np.int32)
    unmasked_indices = np.asarray(unmasked_indices, np.int32)
    Wr, br = np.asarray(Wr, np.float32), np.asarray(br, np.float32)
    W1, b1 = np.asarray(W1, np.float32), np.asarray(b1, np.float32)
    W2, b2 = np.asarray(W2, np.float32), np.asarray(b2, np.float32)
    assert int(range_r) == R and h_L.shape == (B, S, D)

    # ---- host: gathers, masks, routing/combine softmaxes ----
    offs = np.concatenate([np.arange(-R, 0), np.arange(1, R + 1)])  # [W]
    a = mask_indices                                                # [B,M]
    t = a[:, :, None] + offs[None, None, :]                         # [B,M,W]
    in_range = (t >= 0) & (t < S)
    tcl = np.clip(t, 0, S - 1)
    is_un = np.zeros((B, S), bool)
    is_un[np.arange(B)[:, None], unmasked_indices] = True
    valid = in_range & is_un[np.arange(B)[:, None, None], tcl]      # [B,M,W]

    bidx = np.arange(B)[:, None]
    h_mask = h_L[bidx, a]                                           # [B,M,D]
    h_anchor = h_L[np.arange(B)[:, None, None], tcl]                # [B,M,W,D]

    wr = _softmax(h_mask @ Wr + br, axis=-1)                        # [B,M,K]
    scores = np.einsum("bmwd,bmd->bmw", h_anchor, h_mask) / np.sqrt(
        np.float32(D)
    )
    scores = np.where(valid, scores, np.float32(-1e30))
    cw = _softmax(scores, axis=-1) * valid                          # [B,M,W]

    # ---- build per-core shards ----
    # W1 split + pre-transposed chunk layouts.
    W1a = W1[:, :D, :]                                              # [K,D,D4]
    W1b = W1[:, D:, :]
    # ht-major columns (ht*1024 + c*128 + h) so expert-0's first h-tile
    # needs only the first half of the w1a transfer.
    w1a_l = np.ascontiguousarray(
        W1a.reshape(K, CD, 128, 2, 128).transpose(0, 2, 3, 1, 4)
    ).astype(BF16)                                                  # [K,128,2,CD,128]
    w2_l = np.ascontiguousarray(
        W2.reshape(K, 2, 128, D).transpose(0, 2, 1, 3)
    ).astype(BF16)                                                  # [K,128,2,D]
    # Bm = h_mask @ W1b + b1 computed on host (~3% of FLOPs), saves device
    # matmuls.  [B,M,K,D4]
    Bm_h = np.einsum("bmd,kdh->bmkh", h_mask, W1b) + b1[None, None]

    in_maps = []
    for c in range(NCORES):
        b = c // 2
        ms = (c % 2) * MC
        ha_c = h_anchor[b, ms : ms + MC]                            # [MC,W,D]
        # tokens w-major: [W,MC,D] -> [T,D] -> transpose [D,T]
        xaT = ha_c.transpose(1, 0, 2).reshape(T, D).T               # [D,T]
        xa_l = np.ascontiguousarray(
            xaT.reshape(CD, 128, T).transpose(1, 0, 2)
        ).astype(BF16)                                              # [128,CD,T]
        # Bm^T per expert/h-tile, replicated x2 along free to match the
        # 512-wide (two w-group) blocks: [K,128,2,512]
        bm_c = Bm_h[b, ms : ms + MC]                                # [MC,K,D4]
        bmT = bm_c.transpose(1, 2, 0).reshape(K, 2, 128, MC)        # [K,ht,128,MC]
        bm_l = np.ascontiguousarray(
            np.broadcast_to(
                bmT.transpose(0, 2, 1, 3)[:, :, :, None, :],
                (K, 128, 2, 2, MC),
            ).reshape(K, 128, 2, 512)
        ).astype(BF16)
        # router weights per expert, partition-broadcast: [K,128,256]
        wr_c = wr[b, ms : ms + MC]                                  # [MC,K]
        wr_l = np.broadcast_to(
            wr_c.T[:, None, :], (K, 128, MC)
        ).astype(BF16)
        # combine weights w-major, partition-broadcast: [128, T]
        cw_c = cw[b, ms : ms + MC]                                  # [MC,W]
        cwb_l = np.broadcast_to(
            cw_c.T.reshape(1, T), (128, T)
        ).astype(BF16)
        wpk_l = np.concatenate(
            [
                w1a_l.reshape(K, 128, CD * D4),
                bm_l.reshape(K, 128, 1024),
                w2_l.reshape(K, 128, 2 * D),
                wr_l,
            ],
            axis=2,
        )
        in_maps.append(dict(xa=xa_l, wpk=wpk_l, cwb=np.ascontiguousarray(cwb_l)))

    key = "nc"
    if key not in _COMPILED:
        _COMPILED[key] = _build_nc()
    nc = _COMPILED[key]

    res = run_bass_kernel_spmd(
        nc, in_maps, core_ids=list(range(NCORES)), trace=TRACE
    )
    LAST_RESULT = res

    # ---- host: unshard + b2 correction + scatter ----
    delta_h = np.zeros((B, S, D), np.float32)
    sw = cw.sum(-1)                                                 # [B,M]
    corr = (sw[:, :, None] * (wr @ b2)).astype(np.float32)          # [B,M,D]
    for c in range(NCORES):
        b = c // 2
        ms = (c % 2) * MC
        o = res.results[c]["out"]                                   # [128, 8*MC]
        dT = o.reshape(128, 8, MC).transpose(1, 0, 2).reshape(D, MC)
        delta = dT.T + corr[b, ms : ms + MC]                        # [MC,D]
        delta_h[b, a[b, ms : ms + MC]] = delta
    return delta_h


# revision 22
# speedup vs baseline: 1.5940x; 1.0684x over previous
"""AMIP router kernel for 8 TRN2 NeuronCores (Bass/Tile, SPMD data-parallel).

Strategy
--------
B*M = 2048 masked positions are sharded 256 per core (batch-major), weights
replicated, zero collectives.  Routing softmaxes / combine weights / gathers
and the small Bm = hm@W1b + b1 term (<4% of FLOPs combined) run on host; the
device runs the heavy expert MLPs over all 2560 tokens/core.

Since W2 is linear, the neighbor-window sum moves BEFORE matmul-2:

    G_i[h, m]   = wr_i[m] * sum_w cw[m, w] * gelu(ha_{m,w}@W1a_i + Bm_{m,i})
    delta^T     = sum_i W2_i^T @ G_i

so matmul-2 runs on 256 columns per (expert, ht) instead of 2560 -- 10x
less PE work than the baseline that accumulated all per-token products in
PSUM.  The expert-independent combine weight cw scales each token on DVE,
Pool (otherwise idle) accumulates the 10 w-groups into a [128, 512]
running sum (two w-parity halves), and DVE folds + scales by the router
weight wr_i into the [128, 256] matmul-2 rhs.  s = cw*wr factoring also
drops the per-expert broadcast of s from the input stream (-4.6MB DMA).

Layouts are feature-major ([feature_partition, token_free]) so both matmuls
chain without transposes.  Compute dtype bf16 (fp32 PSUM accumulate).

This walrus build enforces tiny per-instruction sync-wait budgets (DVE
tensor ops and 3-source activations: ONE wait; 2-source ACT copies and
matmuls: two; DMAs: one engine wait).  The kernel is choreographed to that
budget: per-engine program-order chaining via ordering-only dep edges, tiny
DVE "observer" copies that advance each engine's observed vector clock so
Tile elides all but one wait per op, all input tiles SBUF-resident, and a
patched kernel-tail drain split into single-wait drains.
"""

import sys

for _p in ("/opt/trn_rl_repo",):
    if _p not in sys.path:
        sys.path.insert(0, _p)

import numpy as np
import ml_dtypes

# Problem constants (hardcoded per task spec).
B, S, D, M, K, R = 4, 2048, 1024, 512, 8, 5
W = 2 * R                 # neighbor window size (10)
D4 = D // 4               # expert hidden (256)
NCORES = 8
MC = (B * M) // NCORES    # masked positions per core (256)
T = W * MC                # device tokens per core (2560), w-major order
NBLK = 5                  # 512-wide token blocks per h-tile (T/512)
CD = D // 128             # contraction chunks over D (8)
W1A_OFF, BM_OFF, W2_OFF, WR_OFF = 0, 2048, 3072, 5120
PK = WR_OFF + 256         # packed per-expert columns (5376)
BF16 = ml_dtypes.bfloat16

_COMPILED = {}            # cache: built Bass graph (shape-only, no data baked)
LAST_RESULT = None        # BassKernelResults of the most recent run
TRACE = False             # set True (e.g. from test.py) to profile


def _patch_tail_drain():
    """Split Tile's kernel-tail drain into several drains with <=4 sem waits
    each -- this walrus build rejects the single many-wait drain the stock
    _drain_and_barrier emits for a kernel touching all 8 HW DMA queues."""
    import concourse.tile as tile
    from concourse.vector_clock import ScopedClock, VectorClock

    if getattr(tile.TileContext, "_tail_drain_patched", False):
        return

    def _drain_and_barrier(self, tick_clock, wait_clock):
        g = tick_clock.global_clock
        n = len(g)
        ticks = [g[i] for i in range(n)]
        nz = [i for i, t in enumerate(ticks) if t > 0]
        CH = 1
        for j in range(0, len(nz), CH):
            keep = set(nz[j : j + CH])
            sub = VectorClock([ticks[i] if i in keep else 0 for i in range(n)])
            d = self.nc.sync.drain()
            wait_clock.add_sem_waits(d.ins, ScopedClock({None: sub}))
        if not nz:
            d = self.nc.sync.drain()
            wait_clock.add_sem_waits(
                d.ins, ScopedClock({None: tick_clock.global_clock})
            )
        self.nc.all_engine_barrier()
        assert self.sems is not None
        popped = self.nc._tile_sem_poison_stack.pop()
        assert popped is self._sem_poison
        self.nc.clear_and_free_semaphores(list(self.sems.allocated().values()))
        self.nc.all_engine_barrier()

    tile.TileContext._drain_and_barrier = _drain_and_barrier
    tile.TileContext._tail_drain_patched = True


def _build_nc():
    import concourse.bass as bass
    import concourse.mybir as mybir
    import concourse.tile as tile
    from contextlib import ExitStack

    _patch_tail_drain()

    bf = mybir.dt.bfloat16
    f32 = mybir.dt.float32
    AF = mybir.ActivationFunctionType

    nc = bass.Bass()
    # DRAM parameters (per-core shards; all pre-laid-out [partition, free]).
    xa = nc.declare_dram_parameter("xa", [128, CD, T], bf, isOutput=False)
    # packed per-expert: [w1a (CD*D4) | bm (2*512) | w2 (2*D) | wr (256)]
    wpk = nc.declare_dram_parameter("wpk", [K, 128, PK], bf, isOutput=False)
    # combine weights, partition-broadcast, token (w-major) on free
    cwb = nc.declare_dram_parameter("cwb", [128, T], bf, isOutput=False)
    out = nc.declare_dram_parameter("out", [128, 8 * MC], f32, isOutput=True)

    with ExitStack() as ctx:
        tc = ctx.enter_context(tile.TileContext(nc))
        const = ctx.enter_context(tc.tile_pool(name="const", bufs=1))
        work = ctx.enter_context(tc.tile_pool(name="work", bufs=2))
        pd = ctx.enter_context(tc.tile_pool(name="pd", bufs=1, space="PSUM"))
        ph = ctx.enter_context(tc.tile_pool(name="ph", bufs=3, space="PSUM"))

        # Everything is resident in SBUF for the whole kernel -- no tile-slot
        # reuse for DMA'd inputs.  (Reused DMA slots create WAW deps against
        # the previous DMA's fanned-out HW queues, blowing the per-instruction
        # sync-wait slot budget in walrus.)
        # Per-engine program-order chaining (ordering-only edges): the
        # scheduler otherwise reorders ready instructions, which breaks the
        # carefully sequenced "observed clock" math that keeps every
        # instruction within its ISA struct's sync-wait budget.
        _last = {}

        def chain(instr, eng):
            if instr is None or not hasattr(instr, "ins"):
                return instr
            prev = _last.get(eng)
            if prev is not None:
                tile.add_dep_helper(
                    instr.ins, prev.ins, sync=False, reason="program-order"
                )
            _last[eng] = instr
            return instr

        # Stage xa: the first 512-token slice of every chunk lands first so
        # the first matmul block can start ~10us earlier; the tail follows.
        xa_sb = const.tile([128, CD, T], bf, tag="xa")
        nc.sync.dma_start(xa_sb[:, :, 0:512], xa[:, :, 0:512])
        cwb_sb = const.tile([128, T], bf, tag="cwb")
        # Explicit zero bias for Gelu: a float bias would be lowered to a
        # framework const AP whose init adds a second sync wait -- over the
        # 3-source Activation struct's budget of one.  DVE-owned zeros let
        # the bias dep consolidate with the DVE data dep into one wait.
        zcol = const.tile([128, 1], f32, tag="zcol")
        chain(nc.vector.memset(zcol[:], 0.0), "dve")
        # Self-chained ACT probe: waiting on its own semaphore advances the
        # scalar engine's observed self-clock, so each gelu's WAW wait
        # against the slot-recycled previous gelu is elided (the 3-source
        # Activation struct only has one sync-wait slot, needed for DVE).
        dummy_act = const.tile([1, 1], f32, tag="dummy_act")
        chain(nc.vector.memset(dummy_act[:], 0.0), "dve")
        # Warm the gelu activation-table load (~2.7us) during the input DMA
        # window instead of on the first real gelu.
        warm_t = const.tile([1, 1], f32, tag="warm_t")
        chain(
            nc.scalar.activation(
                warm_t[:], zcol[0:1, :], AF.Gelu, bias=zcol[0:1, :]
            ),
            "act",
        )
        # DVE observer scratch: tiny copies that advance VectorE's observed
        # clocks of other engines so real DVE ops carry a single sync wait
        # (this walrus build allows only ONE wait on DVE TT/Copy structs).
        scr1 = const.tile([1, 1], bf, tag="scr1")
        scr2 = const.tile([1, 512], bf, tag="scr2")
        scr2e = const.tile([1, 512], bf, tag="scr2e")
        scrp = const.tile([1, 1], bf, tag="scrp")
        # PE warm-up: ~20 rank-1 matmuls (~5us of PE activity) during the
        # input-DMA window keep the HAM clock gate from starting the real
        # matmul stream at half rate.  Dedicated source tile so no real
        # consumer inherits a WAR dep against the warm matmuls.
        warm_src = const.tile([1, 512], bf, tag="warm_src")
        chain(nc.vector.memset(warm_src[:], 0.0), "dve")
        warm_ps = pd.tile([128, 512], f32, tag="warm_ps", name="warm_ps")
        for wk in range(20):
            chain(nc.tensor.matmul(
                warm_ps[:],
                warm_src[0:1, 0:128],
                warm_src[0:1, :],
                start=(wk == 0),
                stop=(wk == 19),
                skip_group_check=True,
            ), "pe")

        # PE "touch" matmuls: rank-1 reads of a freshly DMA'd region that
        # carry the DMA-queue wait on a throwaway instruction, advancing the
        # PE's observed queue clock so the real matmuls (which also need a
        # DVE slot-WAR wait) stay within the single-wait Matmult budget.
        # They overwrite a corner of the (finished) warm-up bank -- a
        # dedicated PSUM tile would cost the bank that hid_ps triple
        # buffering needs.
        def touch(region):
            chain(nc.tensor.matmul(
                warm_ps[0:1, 0:1],
                region,
                region,
                start=True,
                stop=True,
                skip_group_check=True,
            ), "pe")

        # Input stream ordered by first-use time on the critical path: the
        # aggregate DMA bandwidth is the startup gate, so late-needed chunks
        # (cwb tail, wr, w2, experts 1-7) queue after the xa slices expert 0
        # consumes in its first two groups.
        wpk_all = const.tile([128, K, PK], bf, tag="wpk_all")

        def wpk0(lo, hi):
            nc.sync.dma_start(wpk_all[:, 0, lo:hi], wpk[0, :, lo:hi])

        def xa_slice(blk):
            nc.sync.dma_start(
                xa_sb[:, :, blk * 512 : blk * 512 + 512],
                xa[:, :, blk * 512 : blk * 512 + 512],
            )

        wpk0(W1A_OFF, W1A_OFF + 1024)
        wpk0(BM_OFF, W2_OFF)
        # Combine weights land early (ACT stages them into per-block tiles:
        # DVE instructions cannot carry a DMA-queue wait in this walrus
        # build, so every DVE-read tile needs an ACT/DVE producer).
        nc.sync.dma_start(cwb_sb[:, 0:512], cwb[:, 0:512])
        xa_slice(1)
        wpk0(W1A_OFF + 1024, BM_OFF)
        nc.sync.dma_start(cwb_sb[:, 512:T], cwb[:, 512:T])
        wpk0(WR_OFF, PK)
        xa_slice(2)
        xa_slice(3)
        xa_slice(4)
        wpk0(W2_OFF, WR_OFF)
        for i in range(1, K):
            nc.sync.dma_start(wpk_all[:, i], wpk[i])

        # Stage combine weights into ACT-produced per-block tiles.
        cw_st = const.tile([128, NBLK, 512], bf, tag="cw_st")
        for blk in range(NBLK):
            chain(
                nc.scalar.copy(
                    cw_st[:, blk, :], cwb_sb[:, blk * 512 : blk * 512 + 512]
                ),
                "act",
            )

        # Output accumulator in PSUM: delta^T [1024, 256] as 4 banks of
        # [128, 512], each holding two 128-row d-chunks side by side.
        delta_ps = [
            pd.tile([128, 512], f32, tag=f"d{j}", name=f"delta_ps{j}")
            for j in range(4)
        ]

        # Software-pipeline matmul2 one group behind so the PE never
        # head-of-line blocks on the V->S->V->Pool reduction chain.
        pending = []  # [(i, ht, G_tile)]

        def emit_mm2(p):
            i_, ht_, g_ = p
            w2_p = wpk_all[:, i_]
            first = i_ == 0 and ht_ == 0
            last = i_ == K - 1 and ht_ == 1
            for dt in range(8):
                sl = delta_ps[dt // 2][:, (dt % 2) * 256 : (dt % 2) * 256 + 256]
                # start=True clears has_written for the WHOLE BANK, and
                # each bank holds two dt regions -- so only the first
                # region of each bank may issue start.  The second
                # region's first write overwrites (bits cleared by the
                # bank's single start) and accumulates thereafter.
                chain(nc.tensor.matmul(
                    sl,
                    w2_p[:, W2_OFF + ht_ * D + dt * 128 : W2_OFF + ht_ * D + dt * 128 + 128],
                    g_[:],
                    start=(first and dt % 2 == 0),
                    stop=last,
                    skip_group_check=True,
                ), "pe")

        hw_hist = []              # all hw tiles in DVE emission order
        pb_prev = None
        for i in range(K):
            wp = wpk_all[:, i]

            # Bm_i (= hm @ W1b_i + b1_i, already w-replicated) is computed on
            # host (~3% of FLOPs) and staged via ScalarE so the DVE add sees
            # an ACT producer (single-wait budget on DVE TT ops).  Same for
            # the router-weight row wr_i.
            bm_sb = work.tile([128, 1024], bf, tag="bm_sb")
            chain(nc.scalar.copy(bm_sb[:], wp[:, BM_OFF : BM_OFF + 1024]), "act")
            wr_sb = work.tile([128, 256], bf, tag="wr_sb")
            chain(nc.scalar.copy(wr_sb[:], wp[:, WR_OFF : WR_OFF + 256]), "act")

            first_of_expert = True
            for ht in range(2):
                # Pool self-observer: one Pool self-wait on last group's
                # final reduce advances Pool's observed self-clock, so this
                # group's pair-adds carry only their DVE data wait (the
                # recycled-slot WAW would otherwise be a second wait).
                if pb_prev is not None:
                    chain(nc.gpsimd.tensor_copy(scrp[:], pb_prev[0:1, 0:1]), "pool")
                hw_g = []
                pend_mul = []     # [(tmp, blk)] gelu issued, mul not yet

                def emit_mul():
                    """mul for the oldest gelu'd block.  Runs one block
                    behind the bias-add so the V->S->V round-trip (gelu +
                    two sem hops, ~1us) overlaps the next bias-add instead
                    of serializing the per-block pipeline.  The mul carries
                    the gelu wait itself (its only sem wait), which also
                    advances DVE's observed ACT clock for the next bias-add.
                    """
                    tmp_, blk_ = pend_mul.pop(0)
                    hw_t = work.tile([128, 512], bf, tag="hw", bufs=5)
                    chain(nc.vector.tensor_mul(hw_t[:], tmp_[:], cw_st[:, blk_, :]), "dve")
                    hw_g.append(hw_t)
                    hw_hist.append(hw_t)
                    # Pool (otherwise idle) owns the w-window reduction as a
                    # TREE: pair-adds of DVE products carry one DVE wait;
                    # the self-chained combines carry one Pool self-wait.
                    if len(hw_g) == 2:
                        pa_ = work.tile([128, 512], bf, tag="pa")
                        chain(nc.gpsimd.tensor_add(pa_[:], hw_g[0][:], hw_g[1][:]), "pool")
                        hw_g.append(pa_)  # stash (slots 2+ unused for hw)
                    return hw_t

                p_a = p_b = None
                for blk in range(NBLK):
                    # absorb DMA first-touch waits on throwaway touch matmuls
                    if i == 0 and ht == 0:
                        touch(xa_sb[0:1, 0, blk * 512 : blk * 512 + 1])
                    if blk == 0:
                        if i == 0:
                            touch(wp[0:1, W1A_OFF + ht * 1024 : W1A_OFF + ht * 1024 + 1])
                        elif ht == 0:
                            touch(wp[0:1, 0:1])
                    hid_ps = ph.tile([128, 512], f32, tag="hid")
                    for c in range(CD):
                        chain(nc.tensor.matmul(
                            hid_ps[:],
                            wp[:, W1A_OFF + ht * 1024 + c * 128 : W1A_OFF + ht * 1024 + c * 128 + 128],
                            xa_sb[:, c, blk * 512 : blk * 512 + 512],
                            start=(c == 0),
                            stop=(c == CD - 1),
                            skip_group_check=True,
                        ), "pe")
                    if blk == 2 and pending:
                        # one group behind, and two blocks in: the reduction
                        # chain finishes ~3.6us after the previous group's
                        # last mm1, so emitting here keeps the PE stall-free
                        # (a stall resets the p-state ramp).
                        p = pending.pop(0)
                        if p[0] == 0 and p[1] == 0:
                            # expert-0's w2 arrives as a separate DMA chunk;
                            # its queue wait rides a touch, not the matmul.
                            touch(wpk_all[0:1, 0, W2_OFF : W2_OFF + 1])
                        emit_mm2(p)
                    # obs0: a DVE self-wait on the latest mult advances the
                    # observed self-clock, eliding every older same-engine
                    # WAW/RAW (recycled tmp/scr2 slots etc).
                    if hw_hist:
                        chain(nc.vector.tensor_copy(scr1[:], hw_hist[-1][0:1, 0:1]), "dve")
                    # obs2-early at expert start: the bias-add below reads
                    # this expert's freshly ACT-staged bm tile.  Writes its
                    # own scratch -- sharing scr2 with obs2-late would add a
                    # WAR wait there.
                    if first_of_expert:
                        chain(nc.vector.tensor_copy(scr2e[:], bm_sb[0:1, 0:512]), "dve")
                        first_of_expert = False
                    tmp = work.tile([128, 512], bf, tag="tmp", bufs=3)
                    chain(nc.vector.tensor_add(tmp[:], hid_ps[:], bm_sb[:, ht * 512 : ht * 512 + 512]), "dve")
                    # probe: glues ACT to this iteration; its DVE wait makes
                    # the in-place gelu need no further waits.
                    chain(nc.scalar.mul(dummy_act[:], tmp[0:1, 0:1], 0.0), "act")
                    chain(nc.scalar.activation(tmp[:], tmp[:], AF.Gelu, bias=zcol[:]), "act")
                    pend_mul.append((tmp, blk))
                    if len(pend_mul) == 2:
                        hw_new = emit_mul()
                        if blk == 4:
                            p_b = work.tile([128, 512], bf, tag="pb")
                            chain(nc.gpsimd.tensor_add(p_b[:], hw_g[2 + 1][:], hw_new[:]), "pool")
                # tail: the final block's mul, then the combine chain.
                hw_last = emit_mul()
                p_a = hw_g[2]     # stashed pair-add of blocks 0+1
                if i == K - 1 and ht == 1:
                    # Final group: no further mm1 work can hide the Pool
                    # chain's ~2.5us latency, so reduce on DVE (~1.3us) --
                    # the last mm2 starts that much sooner.
                    d1 = work.tile([128, 512], bf, tag="f1")
                    chain(nc.vector.tensor_add(d1[:], p_a[:], p_b[:]), "dve")
                    d2 = work.tile([128, 512], bf, tag="f2")
                    chain(nc.vector.tensor_add(d2[:], d1[:], hw_last[:]), "dve")
                    g1 = work.tile([128, 256], bf, tag="g1")
                    chain(nc.vector.tensor_add(g1[:], d2[:, 0:256], d2[:, 256:512]), "dve")
                    g_t = work.tile([128, 256], bf, tag="G")
                    chain(nc.vector.tensor_mul(g_t[:], g1[:], wr_sb[:]), "dve")
                else:
                    # f1 = pa + pb; f2 = f1 + hw4; g1 folds the two w-parity
                    # halves -- all Pool, each one self/DVE wait.
                    f1 = work.tile([128, 512], bf, tag="f1")
                    chain(nc.gpsimd.tensor_add(f1[:], p_a[:], p_b[:]), "pool")
                    # second Pool self-observer: f2 reads f1 (self) AND hw4
                    # (DVE) -- two waits without this; observing f1 here
                    # leaves f2 with only the DVE data wait.
                    chain(nc.gpsimd.tensor_copy(scrp[:], f1[0:1, 0:1]), "pool")
                    f2 = work.tile([128, 512], bf, tag="f2")
                    chain(nc.gpsimd.tensor_add(f2[:], f1[:], hw_last[:]), "pool")
                    g1 = work.tile([128, 256], bf, tag="g1")
                    chain(nc.gpsimd.tensor_add(g1[:], f2[:, 0:256], f2[:, 256:512]), "pool")
                    # scale by the router weight on DVE: G = g1 * wr_i.
                    g_t = work.tile([128, 256], bf, tag="G")
                    chain(nc.vector.tensor_mul(g_t[:], g1[:], wr_sb[:]), "dve")
                pending.append((i, ht, g_t))
                pb_prev = p_b
        # Final group's mm2 interleaved with the PSUM -> SBUF -> DRAM drain:
        # as soon as a delta bank receives its last accumulation, ACT/DVE
        # copy it out and SWDGE ships it while the PE fills the next bank.
        (i_, ht_, g_) = pending.pop(0)
        assert not pending
        w2_p = wpk_all[:, i_]
        delta_sb = const.tile([128, 4, 512], f32, tag="dsb")
        for j in range(4):
            for h in range(2):
                dt = 2 * j + h
                chain(nc.tensor.matmul(
                    delta_ps[j][:, h * 256 : h * 256 + 256],
                    w2_p[:, W2_OFF + ht_ * D + dt * 128 : W2_OFF + ht_ * D + dt * 128 + 128],
                    g_[:],
                    start=False,
                    stop=True,
                    skip_group_check=True,
                ), "pe")
            if j % 2 == 0:
                chain(nc.scalar.copy(delta_sb[:, j, :], delta_ps[j][:]), "act")
            else:
                chain(
                    nc.vector.tensor_copy(delta_sb[:, j, :], delta_ps[j][:]),
                    "dve",
                )
            # SWDGE for outputs: the HW queues all have prior traffic, and a
            # queue-FIFO self-wait + the ACT data wait exceeds the DMA
            # struct's single sync-wait slot.
            chain(nc.gpsimd.dma_start(out[:, j * 512 : j * 512 + 512], delta_sb[:, j, :]), "pool")
    return nc


def _softmax(x, axis=-1):
    x = x - x.max(axis=axis, keepdims=True)
    e = np.exp(x)
    return e / e.sum(axis=axis, keepdims=True)


def kernel(h_L, mask_indices, unmasked_indices, range_r, Wr, br, W1, b1, W2, b2):
    global LAST_RESULT
    from concourse.bass_utils import run_bass_kernel_spmd

    h_L = np.asarray(h_L, np.float32)
    mask_indices = np.asarray(mask_indices, np.int32)
    unmasked_indices = np.asarray(unmasked_indices, np.int32)
    Wr, br = np.asarray(Wr, np.float32), np.asarray(br, np.float32)
    W1, b1 = np.asarray(W1, np.float32), np.asarray(b1, np.float32)
    W2, b2 = np.asarray(W2, np.float32), np.asarray(b2, np.float32)
    assert int(range_r) == R and h_L.shape == (B, S, D)

    # ---- host: gathers, masks, routing/combine softmaxes ----
    offs = np.concatenate([np.arange(-R, 0), np.arange(1, R + 1)])  # [W]
    a = mask_indices                                                # [B,M]
    t = a[:, :, None] + offs[None, None, :]                         # [B,M,W]
    in_range = (t >= 0) & (t < S)
    tcl = np.clip(t, 0, S - 1)
    is_un = np.zeros((B, S), bool)
    is_un[np.arange(B)[:, None], unmasked_indices] = True
    valid = in_range & is_un[np.arange(B)[:, None, None], tcl]      # [B,M,W]

    bidx = np.arange(B)[:, None]
    h_mask = h_L[bidx, a]                                           # [B,M,D]
    h_anchor = h_L[np.arange(B)[:, None, None], tcl]                # [B,M,W,D]

    wr = _softmax(h_mask @ Wr + br, axis=-1)                        # [B,M,K]
    scores = np.einsum("bmwd,bmd->bmw", h_anchor, h_mask) / np.sqrt(
        np.float32(D)
    )
    scores = np.where(valid, scores, np.float32(-1e30))
    cw = _softmax(scores, axis=-1) * valid                          # [B,M,W]

    # ---- build per-core shards ----
    # W1 split + pre-transposed chunk layouts.
    W1a = W1[:, :D, :]                                              # [K,D,D4]
    W1b = W1[:, D:, :]
    # ht-major columns (ht*1024 + c*128 + h) so expert-0's first h-tile
    # needs only the first half of the w1a transfer.
    w1a_l = np.ascontiguousarray(
        W1a.reshape(K, CD, 128, 2, 128).transpose(0, 2, 3, 1, 4)
    ).astype(BF16)                                                  # [K,128,2,CD,128]
    w2_l = np.ascontiguousarray(
        W2.reshape(K, 2, 128, D).transpose(0, 2, 1, 3)
    ).astype(BF16)                                                  # [K,128,2,D]
    # Bm = h_mask @ W1b + b1 computed on host (~3% of FLOPs), saves device
    # matmuls.  [B,M,K,D4]
    Bm_h = np.einsum("bmd,kdh->bmkh", h_mask, W1b) + b1[None, None]

    in_maps = []
    for c in range(NCORES):
        b = c // 2
        ms = (c % 2) * MC
        ha_c = h_anchor[b, ms : ms + MC]                            # [MC,W,D]
        # tokens w-major: [W,MC,D] -> [T,D] -> transpose [D,T]
        xaT = ha_c.transpose(1, 0, 2).reshape(T, D).T               # [D,T]
        xa_l = np.ascontiguousarray(
            xaT.reshape(CD, 128, T).transpose(1, 0, 2)
        ).astype(BF16)                                              # [128,CD,T]
        # Bm^T per expert/h-tile, replicated x2 along free to match the
        # 512-wide (two w-group) blocks: [K,128,2,512]
        bm_c = Bm_h[b, ms : ms + MC]                                # [MC,K,D4]
        bmT = bm_c.transpose(1, 2, 0).reshape(K, 2, 128, MC)        # [K,ht,128,MC]
        bm_l = np.ascontiguousarray(
            np.broadcast_to(
                bmT.transpose(0, 2, 1, 3)[:, :, :, None, :],
                (K, 128, 2, 2, MC),
            ).reshape(K, 128, 2, 512)
        ).astype(BF16)
        # router weights per expert, partition-broadcast: [K,128,256]
        wr_c = wr[b, ms : ms + MC]                                  # [MC,K]
        wr_l = np.broadcast_to(
            wr_c.T[:, None, :], (K, 128, MC)
        ).astype(BF16)
        # combine weights w-major, partition-broadcast: [128, T]
        cw_c = cw[b, ms : ms + MC]                                  # [MC,W]
        cwb_l = np.broadcast_to(
            cw_c.T.reshape(1, T), (128, T)
        ).astype(BF16)
        wpk_l = np.concatenate(
            [
                w1a_l.reshape(K, 128, CD * D4),
                bm_l.reshape(K, 128, 1024),
                w2_l.reshape(K, 128, 2 * D),
                wr_l,
            ],
            axis=2,
        )
        in_maps.append(dict(xa=xa_l, wpk=wpk_l, cwb=np.ascontiguousarray(cwb_l)))

    key = "nc"
    if key not in _COMPILED:
        _COMPILED[key] = _build_nc()
    nc = _COMPILED[key]

    res = run_bass_kernel_spmd(
        nc, in_maps, core_ids=list(range(NCORES)), trace=TRACE
    )
    LAST_RESULT = res

    # ---- host: unshard + b2 correction + scatter ----
    delta_h = np.zeros((B, S, D), np.float32)
    sw = cw.sum(-1)                                                 # [B,M]
    corr = (sw[:, :, None] * (wr @ b2)).astype(np.float32)          # [B,M,D]
    for c in range(NCORES):
        b = c // 2
        ms = (c % 2) * MC
        o = res.results[c]["out"]                                   # [128, 8*MC]
        dT = o.reshape(128, 8, MC).transpose(1, 0, 2).reshape(D, MC)
        delta = dT.T + corr[b, ms : ms + MC]                        # [MC,D]
        delta_h[b, a[b, ms : ms + MC]] = delta
    return delta_h


# revision 36
# speedup vs baseline: 1.6802x; 1.0540x over previous
"""AMIP router kernel for 8 TRN2 NeuronCores (Bass/Tile, SPMD data-parallel).

Strategy
--------
B*M = 2048 masked positions are sharded 256 per core (batch-major), weights
replicated, zero collectives.  Routing softmaxes / combine weights / gathers
and the small Bm = hm@W1b + b1 term (<4% of FLOPs combined) run on host; the
device runs the heavy expert MLPs over all 2560 tokens/core.

Since W2 is linear, the neighbor-window sum moves BEFORE matmul-2:

    G_i[h, m]   = wr_i[m] * sum_w cw[m, w] * gelu(ha_{m,w}@W1a_i + Bm_{m,i})
    delta^T     = sum_i W2_i^T @ G_i

so matmul-2 runs on 256 columns per (expert, ht) instead of 2560 -- 10x
less PE work than the baseline that accumulated all per-token products in
PSUM.  The expert-independent combine weight cw scales each token on DVE,
Pool (otherwise idle) accumulates the 10 w-groups into a [128, 512]
running sum (two w-parity halves), and DVE folds + scales by the router
weight wr_i into the [128, 256] matmul-2 rhs.  s = cw*wr factoring also
drops the per-expert broadcast of s from the input stream (-4.6MB DMA).

Layouts are feature-major ([feature_partition, token_free]) so both matmuls
chain without transposes.  Compute dtype bf16 (fp32 PSUM accumulate).

This walrus build enforces tiny per-instruction sync-wait budgets (DVE
tensor ops and 3-source activations: ONE wait; 2-source ACT copies and
matmuls: two; DMAs: one engine wait).  The kernel is choreographed to that
budget: per-engine program-order chaining via ordering-only dep edges, tiny
DVE "observer" copies that advance each engine's observed vector clock so
Tile elides all but one wait per op, all input tiles SBUF-resident, and a
patched kernel-tail drain split into single-wait drains.
"""

import sys

for _p in ("/opt/trn_rl_repo",):
    if _p not in sys.path:
        sys.path.insert(0, _p)

import numpy as np
import ml_dtypes

# Problem constants (hardcoded per task spec).
B, S, D, M, K, R = 4, 2048, 1024, 512, 8, 5
W = 2 * R                 # neighbor window size (10)
D4 = D // 4               # expert hidden (256)
NCORES = 8
MC = (B * M) // NCORES    # masked positions per core (256)
T = W * MC                # device tokens per core (2560), w-major order
NBLK = 5                  # 512-wide token blocks per h-tile (T/512)
CD = D // 128             # contraction chunks over D (8)
W1A_OFF, BM_OFF, W2_OFF, WR_OFF = 0, 2048, 3072, 5120
PK = WR_OFF + 256         # packed per-expert columns (5376)
BF16 = ml_dtypes.bfloat16

_COMPILED = {}            # cache: built Bass graph (shape-only, no data baked)
LAST_RESULT = None        # BassKernelResults of the most recent run
TRACE = False             # set True (e.g. from test.py) to profile


def _patch_tail_drain():
    """Split Tile's kernel-tail drain into several drains with <=4 sem waits
    each -- this walrus build rejects the single many-wait drain the stock
    _drain_and_barrier emits for a kernel touching all 8 HW DMA queues."""
    import concourse.tile as tile
    from concourse.vector_clock import ScopedClock, VectorClock

    if getattr(tile.TileContext, "_tail_drain_patched", False):
        return

    def _drain_and_barrier(self, tick_clock, wait_clock):
        g = tick_clock.global_clock
        n = len(g)
        ticks = [g[i] for i in range(n)]
        nz = [i for i, t in enumerate(ticks) if t > 0]
        CH = 1
        for j in range(0, len(nz), CH):
            keep = set(nz[j : j + CH])
            sub = VectorClock([ticks[i] if i in keep else 0 for i in range(n)])
            d = self.nc.sync.drain()
            wait_clock.add_sem_waits(d.ins, ScopedClock({None: sub}))
        if not nz:
            d = self.nc.sync.drain()
            wait_clock.add_sem_waits(
                d.ins, ScopedClock({None: tick_clock.global_clock})
            )
        self.nc.all_engine_barrier()
        assert self.sems is not None
        popped = self.nc._tile_sem_poison_stack.pop()
        assert popped is self._sem_poison
        self.nc.clear_and_free_semaphores(list(self.sems.allocated().values()))
        self.nc.all_engine_barrier()

    tile.TileContext._drain_and_barrier = _drain_and_barrier
    tile.TileContext._tail_drain_patched = True


def _build_nc():
    import concourse.bass as bass
    import concourse.mybir as mybir
    import concourse.tile as tile
    from contextlib import ExitStack

    _patch_tail_drain()

    bf = mybir.dt.bfloat16
    f32 = mybir.dt.float32
    AF = mybir.ActivationFunctionType

    nc = bass.Bass()
    # DRAM parameters (per-core shards; all pre-laid-out [partition, free]).
    xa = nc.declare_dram_parameter("xa", [128, CD, T], bf, isOutput=False)
    # packed per-expert: [w1a (CD*D4) | bm (2*512) | w2 (2*D) | wr (256)]
    wpk = nc.declare_dram_parameter("wpk", [K, 128, PK], bf, isOutput=False)
    # combine weights, single row, token (w-major) on free -- broadcast to
    # 128 partitions on-device by a ones-column matmul (5KB of DMA instead
    # of 640KB on the bandwidth-bound startup path)
    cwb = nc.declare_dram_parameter("cwb", [1, T], bf, isOutput=False)
    out = nc.declare_dram_parameter("out", [128, 8 * MC], f32, isOutput=True)

    with ExitStack() as ctx:
        tc = ctx.enter_context(tile.TileContext(nc))
        const = ctx.enter_context(tc.tile_pool(name="const", bufs=1))
        work = ctx.enter_context(tc.tile_pool(name="work", bufs=2))
        pd = ctx.enter_context(tc.tile_pool(name="pd", bufs=1, space="PSUM"))
        ph = ctx.enter_context(tc.tile_pool(name="ph", bufs=3, space="PSUM"))

        # Everything is resident in SBUF for the whole kernel -- no tile-slot
        # reuse for DMA'd inputs.  (Reused DMA slots create WAW deps against
        # the previous DMA's fanned-out HW queues, blowing the per-instruction
        # sync-wait slot budget in walrus.)
        # Per-engine program-order chaining (ordering-only edges): the
        # scheduler otherwise reorders ready instructions, which breaks the
        # carefully sequenced "observed clock" math that keeps every
        # instruction within its ISA struct's sync-wait budget.
        _last = {}

        def chain(instr, eng):
            if instr is None or not hasattr(instr, "ins"):
                return instr
            prev = _last.get(eng)
            if prev is not None:
                tile.add_dep_helper(
                    instr.ins, prev.ins, sync=False, reason="program-order"
                )
            _last[eng] = instr
            return instr

        # Stage xa: the first 512-token slice of every chunk lands first so
        # the first matmul block can start ~10us earlier; the tail follows.
        xa_sb = const.tile([128, CD, T], bf, tag="xa")
        cwb_sb = const.tile([1, T], bf, tag="cwb")
        nc.sync.dma_start(cwb_sb[:], cwb[:])
        nc.sync.dma_start(xa_sb[:, :, 0:512], xa[:, :, 0:512])
        # Explicit zero bias for Gelu: a float bias would be lowered to a
        # framework const AP whose init adds a second sync wait -- over the
        # 3-source Activation struct's budget of one.  DVE-owned zeros let
        # the bias dep consolidate with the DVE data dep into one wait.
        zcol = const.tile([128, 1], f32, tag="zcol")
        chain(nc.vector.memset(zcol[:], 0.0), "dve")
        # Self-chained ACT probe: waiting on its own semaphore advances the
        # scalar engine's observed self-clock, so each gelu's WAW wait
        # against the slot-recycled previous gelu is elided (the 3-source
        # Activation struct only has one sync-wait slot, needed for DVE).
        dummy_act = const.tile([1, 1], f32, tag="dummy_act")
        chain(nc.vector.memset(dummy_act[:], 0.0), "dve")
        # Warm the gelu activation-table load (~2.7us) during the input DMA
        # window instead of on the first real gelu.
        warm_t = const.tile([1, 1], f32, tag="warm_t")
        chain(
            nc.scalar.activation(
                warm_t[:], zcol[0:1, :], AF.Gelu, bias=zcol[0:1, :]
            ),
            "act",
        )
        # DVE observer scratch: tiny copies that advance VectorE's observed
        # clocks of other engines so real DVE ops carry a single sync wait
        # (this walrus build allows only ONE wait on DVE TT/Copy structs).
        scr1 = const.tile([1, 1], bf, tag="scr1")
        scr2 = const.tile([1, 512], bf, tag="scr2")
        scr2e = const.tile([1, 512], bf, tag="scr2e")
        scrp = const.tile([1, 1], bf, tag="scrp")
        # PE warm-up: ~20 rank-1 matmuls (~5us of PE activity) during the
        # input-DMA window keep the HAM clock gate from starting the real
        # matmul stream at half rate.  Dedicated source tile so no real
        # consumer inherits a WAR dep against the warm matmuls.
        warm_src = const.tile([1, 512], bf, tag="warm_src")
        chain(nc.vector.memset(warm_src[:], 0.0), "dve")
        ones_t = const.tile([1, 128], bf, tag="ones_t")
        chain(nc.vector.memset(ones_t[:], 1.0), "dve")
        warm_ps = pd.tile([128, 512], f32, tag="warm_ps", name="warm_ps")
        for wk in range(20):
            chain(nc.tensor.matmul(
                warm_ps[:],
                warm_src[0:1, 0:128],
                warm_src[0:1, :],
                start=(wk == 0),
                stop=(wk == 19),
                skip_group_check=True,
            ), "pe")

        # PE "touch" matmuls: rank-1 reads of a freshly DMA'd region that
        # carry the DMA-queue wait on a throwaway instruction, advancing the
        # PE's observed queue clock so the real matmuls (which also need a
        # DVE slot-WAR wait) stay within the single-wait Matmult budget.
        # They overwrite a corner of the (finished) warm-up bank -- a
        # dedicated PSUM tile would cost the bank that hid_ps triple
        # buffering needs.
        def touch(region):
            chain(nc.tensor.matmul(
                warm_ps[0:1, 0:1],
                region,
                region,
                start=True,
                stop=True,
                skip_group_check=True,
            ), "pe")

        # Input stream ordered by first-use time on the critical path: the
        # aggregate DMA bandwidth is the startup gate, so late-needed chunks
        # (cwb tail, wr, w2, experts 1-7) queue after the xa slices expert 0
        # consumes in its first two groups.
        wpk_all = const.tile([128, K, PK], bf, tag="wpk_all")

        def wpk0(lo, hi):
            nc.sync.dma_start(wpk_all[:, 0, lo:hi], wpk[0, :, lo:hi])

        def xa_slice(blk):
            nc.sync.dma_start(
                xa_sb[:, :, blk * 512 : blk * 512 + 512],
                xa[:, :, blk * 512 : blk * 512 + 512],
            )

        wpk0(W1A_OFF, W1A_OFF + 1024)
        wpk0(BM_OFF, W2_OFF)
        xa_slice(1)
        wpk0(W1A_OFF + 1024, BM_OFF)
        wpk0(WR_OFF, PK)
        xa_slice(2)
        xa_slice(3)
        xa_slice(4)
        wpk0(W2_OFF, WR_OFF)
        for i in range(1, K):
            nc.sync.dma_start(wpk_all[:, i], wpk[i])

        # Broadcast combine weights to all partitions via ones-column
        # matmuls (in the idle slot right after warm-up), then ACT stages
        # them into per-block tiles: DVE instructions cannot carry a
        # DMA-queue wait in this walrus build, so every DVE-read tile
        # needs an ACT/DVE/PE-observable producer.
        cw_st = const.tile([128, NBLK, 512], bf, tag="cw_st")
        touch(cwb_sb[0:1, 0:1])
        for blk in range(NBLK):
            bc_ps = ph.tile([128, 512], f32, tag="hid")
            chain(nc.tensor.matmul(
                bc_ps[:],
                ones_t[:],
                cwb_sb[0:1, blk * 512 : blk * 512 + 512],
                start=True,
                stop=True,
                skip_group_check=True,
            ), "pe")
            chain(nc.scalar.copy(cw_st[:, blk, :], bc_ps[:]), "act")

        # Output accumulator in PSUM: delta^T [1024, 256] as 4 banks of
        # [128, 512], each holding two 128-row d-chunks side by side.
        delta_ps = [
            pd.tile([128, 512], f32, tag=f"d{j}", name=f"delta_ps{j}")
            for j in range(4)
        ]

        # Software-pipeline matmul2 one group behind so the PE never
        # head-of-line blocks on the V->S->V->Pool reduction chain.
        pending = []  # [(i, ht, G_tile)]

        def emit_mm2(p):
            i_, ht_, g_ = p
            w2_p = wpk_all[:, i_]
            first = i_ == 0 and ht_ == 0
            last = i_ == K - 1 and ht_ == 1
            for dt in range(8):
                sl = delta_ps[dt // 2][:, (dt % 2) * 256 : (dt % 2) * 256 + 256]
                # start=True clears has_written for the WHOLE BANK, and
                # each bank holds two dt regions -- so only the first
                # region of each bank may issue start.  The second
                # region's first write overwrites (bits cleared by the
                # bank's single start) and accumulates thereafter.
                chain(nc.tensor.matmul(
                    sl,
                    w2_p[:, W2_OFF + ht_ * D + dt * 128 : W2_OFF + ht_ * D + dt * 128 + 128],
                    g_[:],
                    start=(first and dt % 2 == 0),
                    stop=last,
                    skip_group_check=True,
                ), "pe")

        hw_hist = []              # all hw tiles in DVE emission order
        pb_prev = None
        deferred_gt = []          # [(g1, wr, i, ht)] Pool reduce done, G not yet

        def emit_gt():
            """G = g1 * wr on DVE, deferred into the NEXT group's stream:
            emitted in the group tail it would make the in-order DVE queue
            wait ~2.5us for the Pool chain, stalling the next group's
            bias-adds (and transitively the PE via the hid-slot WAR)."""
            g1_, wr_, i_, ht_ = deferred_gt.pop(0)
            g_t = work.tile([128, 256], bf, tag="G")
            chain(nc.vector.tensor_mul(g_t[:], g1_[:], wr_[:]), "dve")
            pending.append((i_, ht_, g_t))

        for i in range(K):
            wp = wpk_all[:, i]

            # Bm_i (= hm @ W1b_i + b1_i, already w-replicated) is computed on
            # host (~3% of FLOPs) and staged via ScalarE so the DVE add sees
            # an ACT producer (single-wait budget on DVE TT ops).  Same for
            # the router-weight row wr_i.
            bm_sb = work.tile([128, 1024], bf, tag="bm_sb")
            chain(nc.scalar.copy(bm_sb[:], wp[:, BM_OFF : BM_OFF + 1024]), "act")
            wr_sb = work.tile([128, 256], bf, tag="wr_sb")
            chain(nc.scalar.copy(wr_sb[:], wp[:, WR_OFF : WR_OFF + 256]), "act")

            first_of_expert = True
            for ht in range(2):
                # Pool self-observer: one Pool self-wait on last group's
                # final reduce advances Pool's observed self-clock, so this
                # group's pair-adds carry only their DVE data wait (the
                # recycled-slot WAW would otherwise be a second wait).
                if pb_prev is not None:
                    chain(nc.gpsimd.tensor_copy(scrp[:], pb_prev[0:1, 0:1]), "pool")
                hw_g = []
                pend_mul = []     # [(tmp, blk)] gelu issued, mul not yet
                # the last group reduces on DVE: no further mm1 hides the
                # Pool chain's latency there, and DVE pair-adds start as
                # soon as each product lands.
                pool_reduce = not (i == K - 1 and ht == 1)
                e_pair = []

                def emit_mul():
                    """mul for the oldest gelu'd block.  Runs one block
                    behind the bias-add so the V->S->V round-trip (gelu +
                    two sem hops, ~1us) overlaps the next bias-add instead
                    of serializing the per-block pipeline.  The mul carries
                    the gelu wait itself (its only sem wait), which also
                    advances DVE's observed ACT clock for the next bias-add.
                    """
                    tmp_, blk_ = pend_mul.pop(0)
                    hw_t = work.tile([128, 512], bf, tag="hw", bufs=5)
                    chain(nc.vector.tensor_mul(hw_t[:], tmp_[:], cw_st[:, blk_, :]), "dve")
                    hw_g.append(hw_t)
                    hw_hist.append(hw_t)
                    # Pool (otherwise idle) owns the w-window reduction as a
                    # TREE: pair-adds of DVE products carry one DVE wait;
                    # the self-chained combines carry one Pool self-wait.
                    if len(hw_g) == 2 and pool_reduce:
                        pa_ = work.tile([128, 512], bf, tag="pa")
                        chain(nc.gpsimd.tensor_add(pa_[:], hw_g[0][:], hw_g[1][:]), "pool")
                        hw_g.append(pa_)  # stash (slots 2+ unused for hw)
                    elif len(hw_g) == 2:
                        e1_ = work.tile([128, 512], bf, tag="e1", bufs=1)
                        chain(nc.vector.tensor_add(e1_[:], hw_g[0][:], hw_g[1][:]), "dve")
                        e_pair.append(e1_)
                    elif len(hw_g) == 4 and not pool_reduce:
                        e2_ = work.tile([128, 512], bf, tag="e2", bufs=1)
                        chain(nc.vector.tensor_add(e2_[:], hw_g[2][:], hw_g[3][:]), "dve")
                        e_pair.append(e2_)
                    return hw_t

                p_a = p_b = None
                for blk in range(NBLK):
                    # absorb DMA first-touch waits on throwaway touch matmuls
                    if i == 0 and ht == 0:
                        touch(xa_sb[0:1, 0, blk * 512 : blk * 512 + 1])
                    if blk == 0:
                        if i == 0:
                            touch(wp[0:1, W1A_OFF + ht * 1024 : W1A_OFF + ht * 1024 + 1])
                        elif ht == 0:
                            touch(wp[0:1, 0:1])
                    hid_ps = ph.tile([128, 512], f32, tag="hid")
                    for c in range(CD):
                        chain(nc.tensor.matmul(
                            hid_ps[:],
                            wp[:, W1A_OFF + ht * 1024 + c * 128 : W1A_OFF + ht * 1024 + c * 128 + 128],
                            xa_sb[:, c, blk * 512 : blk * 512 + 512],
                            start=(c == 0),
                            stop=(c == CD - 1),
                            skip_group_check=True,
                        ), "pe")
                    if blk == 3 and pending:
                        # one group behind, and three blocks in: the
                        # reduction chain finishes ~4.3us after the previous
                        # group's last mm1, so emitting here keeps the PE
                        # stall-free (a stall resets the p-state ramp).
                        p = pending.pop(0)
                        if p[0] == 0 and p[1] == 0:
                            # expert-0's w2 arrives as a separate DMA chunk;
                            # its queue wait rides a touch, not the matmul.
                            touch(wpk_all[0:1, 0, W2_OFF : W2_OFF + 1])
                        emit_mm2(p)
                    # obs0: a DVE self-wait on the latest mult advances the
                    # observed self-clock, eliding every older same-engine
                    # WAW/RAW (recycled tmp/scr2 slots etc).
                    if hw_hist:
                        chain(nc.vector.tensor_copy(scr1[:], hw_hist[-1][0:1, 0:1]), "dve")
                    # obs2-early at expert start: the bias-add below reads
                    # this expert's freshly ACT-staged bm tile.  Writes its
                    # own scratch -- sharing scr2 with obs2-late would add a
                    # WAR wait there.
                    if first_of_expert:
                        chain(nc.vector.tensor_copy(scr2e[:], bm_sb[0:1, 0:512]), "dve")
                        first_of_expert = False
                    tmp = work.tile([128, 512], bf, tag="tmp", bufs=3)
                    chain(nc.vector.tensor_add(tmp[:], hid_ps[:], bm_sb[:, ht * 512 : ht * 512 + 512]), "dve")
                    # probe: glues ACT to this iteration; its DVE wait makes
                    # the in-place gelu need no further waits.
                    chain(nc.scalar.mul(dummy_act[:], tmp[0:1, 0:1], 0.0), "act")
                    chain(nc.scalar.activation(tmp[:], tmp[:], AF.Gelu, bias=zcol[:]), "act")
                    pend_mul.append((tmp, blk))
                    if len(pend_mul) == 2:
                        if blk == 1 and deferred_gt:
                            emit_gt()
                        hw_new = emit_mul()
                        if blk == 4 and pool_reduce:
                            p_b = work.tile([128, 512], bf, tag="pb")
                            chain(nc.gpsimd.tensor_add(p_b[:], hw_g[2 + 1][:], hw_new[:]), "pool")
                # tail: the final block's mul, then the combine chain.
                hw_last = emit_mul()
                if not pool_reduce:
                    # Final group: all-DVE tree, pair-adds already emitted
                    # inline -- only ~1.3us of chain remains after the last
                    # mul, so the last mm2 starts ~2.5us sooner than via
                    # the (possibly queued) Pool path.
                    d1 = work.tile([128, 512], bf, tag="f1")
                    chain(nc.vector.tensor_add(d1[:], e_pair[0][:], e_pair[1][:]), "dve")
                    d2 = work.tile([128, 512], bf, tag="f2")
                    chain(nc.vector.tensor_add(d2[:], d1[:], hw_last[:]), "dve")
                    g1 = work.tile([128, 256], bf, tag="g1")
                    chain(nc.vector.tensor_add(g1[:], d2[:, 0:256], d2[:, 256:512]), "dve")
                    g_t = work.tile([128, 256], bf, tag="G")
                    chain(nc.vector.tensor_mul(g_t[:], g1[:], wr_sb[:]), "dve")
                    pending.append((i, ht, g_t))
                else:
                    p_a = hw_g[2]     # stashed pair-add of blocks 0+1
                    # f1 = pa + pb; f2 = f1 + hw4; g1 folds the two w-parity
                    # halves -- all Pool, each one self/DVE wait.
                    f1 = work.tile([128, 512], bf, tag="f1")
                    chain(nc.gpsimd.tensor_add(f1[:], p_a[:], p_b[:]), "pool")
                    # second Pool self-observer: f2 reads f1 (self) AND hw4
                    # (DVE) -- two waits without this; observing f1 here
                    # leaves f2 with only the DVE data wait.
                    chain(nc.gpsimd.tensor_copy(scrp[:], f1[0:1, 0:1]), "pool")
                    f2 = work.tile([128, 512], bf, tag="f2")
                    chain(nc.gpsimd.tensor_add(f2[:], f1[:], hw_last[:]), "pool")
                    g1 = work.tile([128, 256], bf, tag="g1")
                    chain(nc.gpsimd.tensor_add(g1[:], f2[:, 0:256], f2[:, 256:512]), "pool")
                    deferred_gt.append((g1, wr_sb, i, ht))
                pb_prev = p_b
        # Final group's mm2 interleaved with the PSUM -> SBUF -> DRAM drain:
        # as soon as a delta bank receives its last accumulation, ACT/DVE
        # copy it out and SWDGE ships it while the PE fills the next bank.
        (i_, ht_, g_) = pending.pop(0)
        assert not pending
        w2_p = wpk_all[:, i_]
        delta_sb = const.tile([128, 4, 512], f32, tag="dsb")
        for j in range(4):
            for h in range(2):
                dt = 2 * j + h
                chain(nc.tensor.matmul(
                    delta_ps[j][:, h * 256 : h * 256 + 256],
                    w2_p[:, W2_OFF + ht_ * D + dt * 128 : W2_OFF + ht_ * D + dt * 128 + 128],
                    g_[:],
                    start=False,
                    stop=True,
                    skip_group_check=True,
                ), "pe")
            if j % 2 == 0:
                chain(nc.scalar.copy(delta_sb[:, j, :], delta_ps[j][:]), "act")
            else:
                chain(
                    nc.vector.tensor_copy(delta_sb[:, j, :], delta_ps[j][:]),
                    "dve",
                )
            # SWDGE for outputs: the HW queues all have prior traffic, and a
            # queue-FIFO self-wait + the ACT data wait exceeds the DMA
            # struct's single sync-wait slot.
            chain(nc.gpsimd.dma_start(out[:, j * 512 : j * 512 + 512], delta_sb[:, j, :]), "pool")
    return nc


def _softmax(x, axis=-1):
    x = x - x.max(axis=axis, keepdims=True)
    e = np.exp(x)
    return e / e.sum(axis=axis, keepdims=True)


def kernel(h_L, mask_indices, unmasked_indices, range_r, Wr, br, W1, b1, W2, b2):
    global LAST_RESULT
    from concourse.bass_utils import run_bass_kernel_spmd

    h_L = np.asarray(h_L, np.float32)
    mask_indices = np.asarray(mask_indices, np.int32)
    unmasked_indices = np.asarray(unmasked_indices, np.int32)
    Wr, br = np.asarray(Wr, np.float32), np.asarray(br, np.float32)
    W1, b1 = np.asarray(W1, np.float32), np.asarray(b1, np.float32)
    W2, b2 = np.asarray(W2, np.float32), np.asarray(b2, np.float32)
    assert int(range_r) == R and h_L.shape == (B, S, D)

    # ---- host: gathers, masks, routing/combine softmaxes ----
    offs = np.concatenate([np.arange(-R, 0), np.arange(1, R + 1)])  # [W]
    a = mask_indices                                                # [B,M]
    t = a[:, :, None] + offs[None, None, :]                         # [B,M,W]
    in_range = (t >= 0) & (t < S)
    tcl = np.clip(t, 0, S - 1)
    is_un = np.zeros((B, S), bool)
    is_un[np.arange(B)[:, None], unmasked_indices] = True
    valid = in_range & is_un[np.arange(B)[:, None, None], tcl]      # [B,M,W]

    bidx = np.arange(B)[:, None]
    h_mask = h_L[bidx, a]                                           # [B,M,D]
    h_anchor = h_L[np.arange(B)[:, None, None], tcl]                # [B,M,W,D]

    wr = _softmax(h_mask @ Wr + br, axis=-1)                        # [B,M,K]
    scores = np.einsum("bmwd,bmd->bmw", h_anchor, h_mask) / np.sqrt(
        np.float32(D)
    )
    scores = np.where(valid, scores, np.float32(-1e30))
    cw = _softmax(scores, axis=-1) * valid                          # [B,M,W]

    # ---- build per-core shards ----
    # W1 split + pre-transposed chunk layouts.
    W1a = W1[:, :D, :]                                              # [K,D,D4]
    W1b = W1[:, D:, :]
    # ht-major columns (ht*1024 + c*128 + h) so expert-0's first h-tile
    # needs only the first half of the w1a transfer.
    w1a_l = np.ascontiguousarray(
        W1a.reshape(K, CD, 128, 2, 128).transpose(0, 2, 3, 1, 4)
    ).astype(BF16)                                                  # [K,128,2,CD,128]
    w2_l = np.ascontiguousarray(
        W2.reshape(K, 2, 128, D).transpose(0, 2, 1, 3)
    ).astype(BF16)                                                  # [K,128,2,D]
    # Bm = h_mask @ W1b + b1 computed on host (~3% of FLOPs), saves device
    # matmuls.  [B,M,K,D4]
    Bm_h = np.einsum("bmd,kdh->bmkh", h_mask, W1b) + b1[None, None]

    in_maps = []
    for c in range(NCORES):
        b = c // 2
        ms = (c % 2) * MC
        ha_c = h_anchor[b, ms : ms + MC]                            # [MC,W,D]
        # tokens w-major: [W,MC,D] -> [T,D] -> transpose [D,T]
        xaT = ha_c.transpose(1, 0, 2).reshape(T, D).T               # [D,T]
        xa_l = np.ascontiguousarray(
            xaT.reshape(CD, 128, T).transpose(1, 0, 2)
        ).astype(BF16)                                              # [128,CD,T]
        # Bm^T per expert/h-tile, replicated x2 along free to match the
        # 512-wide (two w-group) blocks: [K,128,2,512]
        bm_c = Bm_h[b, ms : ms + MC]                                # [MC,K,D4]
        bmT = bm_c.transpose(1, 2, 0).reshape(K, 2, 128, MC)        # [K,ht,128,MC]
        bm_l = np.ascontiguousarray(
            np.broadcast_to(
                bmT.transpose(0, 2, 1, 3)[:, :, :, None, :],
                (K, 128, 2, 2, MC),
            ).reshape(K, 128, 2, 512)
        ).astype(BF16)
        # router weights per expert, partition-broadcast: [K,128,256]
        wr_c = wr[b, ms : ms + MC]                                  # [MC,K]
        wr_l = np.broadcast_to(
            wr_c.T[:, None, :], (K, 128, MC)
        ).astype(BF16)
        # combine weights w-major, single row (device broadcasts): [1, T]
        cw_c = cw[b, ms : ms + MC]                                  # [MC,W]
        cwb_l = cw_c.T.reshape(1, T).astype(BF16)
        wpk_l = np.concatenate(
            [
                w1a_l.reshape(K, 128, CD * D4),
                bm_l.reshape(K, 128, 1024),
                w2_l.reshape(K, 128, 2 * D),
                wr_l,
            ],
            axis=2,
        )
        in_maps.append(dict(xa=xa_l, wpk=wpk_l, cwb=np.ascontiguousarray(cwb_l)))

    key = "nc"
    if key not in _COMPILED:
        _COMPILED[key] = _build_nc()
    nc = _COMPILED[key]

    res = run_bass_kernel_spmd(
        nc, in_maps, core_ids=list(range(NCORES)), trace=TRACE
    )
    LAST_RESULT = res

    # ---- host: unshard + b2 correction + scatter ----
    delta_h = np.zeros((B, S, D), np.float32)
    sw = cw.sum(-1)                                                 # [B,M]
    corr = (sw[:, :, None] * (wr @ b2)).astype(np.float32)          # [B,M,D]
    for c in range(NCORES):
        b = c // 2
        ms = (c % 2) * MC
        o = res.results[c]["out"]                                   # [128, 8*MC]
        dT = o.reshape(128, 8, MC).transpose(1, 0, 2).reshape(D, MC)
        delta = dT.T + corr[b, ms : ms + MC]                        # [MC,D]
        delta_h[b, a[b, ms : ms + MC]] = delta
    return delta_h
